# revision 38
# baseline (speedup 1.0000x reference)
"""Trainium2 Bass kernel for nn_CausalCrossConditionalSelfAttention.

Data-parallel over batch B=8, one element per core. Design:
  - Exact T=1026 (no padding): query chunks (384,386,256), key blocks
    8x128 + one 2-row tiny block; scores/exp/AV operate on per-block
    column ranges [zlo,zhi) so the causal triangle / local band is not
    padded to full chunk width.
  - bf16 for x/weights/qT/kT/pt/vext/masks (halves DMA, 2x DVE mask-muls,
    any-N matmuls); fp32 psums, Z path, and final out-projection.
  - ~15 DMAs total (each DMACopy costs ~630ns on the shared HWDGE).
  - Softmax denominators ride as a ones-column in the AV matmul; Z rows are
    scaled by 1/mix-factor into a [65,W] staging tile (partitions 0/64),
    broadcast to 128 partitions by one select-matmul per head-pair, and
    applied via reciprocal + in-place multiply (mix weights folded in).
  - Score blocks bin-packed into [128,1024] psum tiles (256-wide slots for
    the 256 chunk, contiguous runs for narrow local-band blocks, stride-512
    pairs for wide blocks) to minimize exp instruction count.
  - One global software-pipelined emission pass: chunk n's score waves
    interleave with chunk n-2's AV matmuls; projections and out-projections
    fill PE slack; per-head-pair normalization fires as soon as both
    members finish.
"""

import sys

if "/opt/trn_rl_repo" not in sys.path:
    sys.path.insert(0, "/opt/trn_rl_repo")

import numpy as np

try:
    import ml_dtypes
    BF16 = np.dtype(ml_dtypes.bfloat16)
    F8 = np.dtype(ml_dtypes.float8_e4m3)
except ImportError:  # pragma: no cover
    BF16 = None
    F8 = None

# fp8 weight scale: w*scale values (~0.0025) sit in e4m3's subnormal range,
# so store w*SW and multiply psum by 1/SW in the evacuation op.
SW = 256.0
DEBUG_TAPS = ()
DEBUG_QG = 1

# ----------------------------------------------------------------------------
# problem constants
# ----------------------------------------------------------------------------
BLOCK = 512
RECEP = 4
N_HEAD = 8
EMBED = 512
HS = 64
T = 2 * BLOCK + 2          # 1026
NSM = 10
NCORES = 8

# query chunks (offset, width): 128-aligned starts so the causal staircase's
# block zlo values never land inside a 128-query group (AV out base always 0)
ICS = [(0, 384), (384, 384), (768, 258)]
# key blocks (offset, height)
JBS = [(j * 128, 128) for j in range(8)] + [(1024, 2)]

# softmax id -> (mask kind, q/k source, v head)
SM_INFO = [
    (0, "loc", "main", 0), (1, "loc", "main", 1),
    (2, "seq", "main", 2), (3, "seq", "main", 3),
    (4, "seq", "main", 4), (5, "seq", "main", 5),
    (6, "seq", "main", 6), (7, "seq", "main", 7),
    (8, "loc", "ml", 2), (9, "loc", "ml", 3),
]
# softmax emission generations per chunk: Y^T psum gen tiles hold 5 softmaxes
# (5*65=325 cols, one bank); ytg staging column position of softmax s
GEN_SMS = [[0, 1, 2, 3, 4], [5, 6, 7, 8, 9]]
POS = {s: gi * 5 + i for gi, g in enumerate(GEN_SMS) for i, s in enumerate(g)}


def chunk_qgroups(ici):
    i0, W = ICS[ici]
    return [(g * 128, min(128, W - g * 128)) for g in range((W + 127) // 128)]


# ----------------------------------------------------------------------------
# host-side plan construction
# ----------------------------------------------------------------------------
def build_perm():
    perm = np.zeros(T, dtype=np.int64)
    perm[0], perm[1] = 0, 1
    b = np.arange(BLOCK)
    perm[2 + 2 * b] = 2 + b
    perm[3 + 2 * b] = 2 + BLOCK + b
    inv = np.argsort(perm)
    return perm, inv


def build_masks_orig():
    to = np.concatenate([np.zeros(2), np.arange(BLOCK) * 2 + 1, np.arange(BLOCK) * 2 + 2])
    seq = to[None, :] <= to[:, None]
    qo = np.concatenate([np.arange(BLOCK) * 2 + 1 - 2 * RECEP + 1] * 2)
    ko = np.concatenate([np.arange(BLOCK) * 2 + 1] * 2)
    de = ko[None, :] < qo[:, None]
    loc = seq.copy()
    loc[2:, 2:] = loc[2:, 2:] & (~de)
    return seq, loc


def build_block_plan():
    """Per (kind, ic): list of block dicts with exact column ranges.

    block = dict(jb, j0, rows, zlo, zhi, bias, mask=(mid,c0,c1) or None)
    Ordered so the first block covers [0, W) (widest) for PSUM start=True.
    """
    perm, _ = build_perm()
    seq, loc = build_masks_orig()
    Ms = seq[perm][:, perm]
    Ml = loc[perm][:, perm]

    mask_tiles = []
    tile_index = {}

    def tile_id(tile):
        key = tile.tobytes() + bytes(str(tile.shape), "ascii")
        if key not in tile_index:
            tile_index[key] = len(mask_tiles)
            mask_tiles.append(tile)
        return tile_index[key]

    plans = {}
    for kind, M in (("seq", Ms), ("loc", Ml)):
        plan = []
        for i0, W in ICS:
            blocks = []
            for jb, (j0, JH) in enumerate(JBS):
                sub = M[i0:i0 + W, j0:j0 + JH].T  # [JH, W] keys x queries
                if not sub.any():
                    continue
                nz_rows = np.flatnonzero(sub.any(axis=1))
                rows = int(nz_rows.max()) + 1
                colmask = sub[:rows].any(axis=0)
                nz_cols = np.flatnonzero(colmask)
                zlo, zhi = int(nz_cols.min()), int(nz_cols.max()) + 1
                core = sub[:rows, zlo:zhi]
                if core.all():
                    mask = None
                else:
                    pc = np.flatnonzero(~core.all(axis=0))
                    c0, c1 = zlo + int(pc.min()), zlo + int(pc.max()) + 1
                    mid = tile_id(
                        sub[:rows, c0:c1].astype(np.float32).copy())
                    mask = (mid, c0, c1)
                blocks.append(dict(jb=jb, j0=j0, rows=rows, zlo=zlo, zhi=zhi,
                                   bias=(j0 == 0), mask=mask))
            # widest-coverage block first (needed for PSUM start=True)
            blocks.sort(key=lambda b: (b["zlo"], -b["zhi"]))
            assert blocks[0]["zlo"] == 0 and blocks[0]["zhi"] == W, (kind, i0)
            plan.append(blocks)
        plans[kind] = plan

    offs, cat = [], []
    o = 0
    for t in mask_tiles:
        offs.append((o, t.shape[1]))
        cat.append(np.pad(t, ((0, 128 - t.shape[0]), (0, 0))))
        o += t.shape[1]
    maskcat = (np.concatenate(cat, axis=1) if cat
               else np.zeros((128, 0), np.float32))
    return plans, maskcat, offs


def build_exp_tiles(blocks, W):
    """Pack a chunk's blocks into [128,1024] score-psum tiles.

    Returns a list of tiles; each tile is a dict:
      placements: [(block, off)]          off in [0,1024), bank-contained
      exps: [("single", block, off)]      bias / tiny blocks
            [("run", [blocks], off, w)]   contiguous narrow blocks, one bank
            [("strided", [blocks], off0, stride, wmax)]
    """
    def bw(b):
        return b["zhi"] - b["zlo"]

    specials = [b for b in blocks if b["bias"] or b["rows"] < 128]
    plain = sorted((b for b in blocks if not (b["bias"] or b["rows"] < 128)),
                   key=lambda b: b["jb"])
    tiles = []

    def new_tile():
        tiles.append(dict(placements=[], exps=[], used=0))
        return tiles[-1]

    if W <= 256:
        # uniform 256-wide slots, 4 per tile; strided exps over plain runs
        slots = specials + plain  # bias first, then jb order
        t = None
        for i, b in enumerate(slots):
            si = i % 4
            if si == 0:
                t = new_tile()
            t["placements"].append((b, si * 256))
        # exps: walk slots; specials single, plain grouped per tile
        for ti, t in enumerate(tiles):
            runb, ro, wmax = [], 0, 0
            for b, off in t["placements"]:
                if b["bias"] or b["rows"] < 128:
                    t["exps"].append(("single", b, off))
                else:
                    if not runb:
                        ro = off
                    runb.append(b)
                    wmax = max(wmax, bw(b))
            if runb:
                t["exps"].append(("strided", runb, ro, 256, wmax))
        return tiles

    wide = [b for b in plain if bw(b) > 256]
    narrow = [b for b in plain if bw(b) <= 256]
    # wide: stride-512 pairs occupying a full tile; narrow leftovers are
    # appended into pair spare bank space, extending the exp width (the
    # shorter bank's tail exps stale psum, which is never read downstream)
    pairs = []
    i = 0
    while i < len(wide):
        t = new_tile()
        pair = wide[i:i + 2]
        ext = []
        for g, b in enumerate(pair):
            t["placements"].append((b, g * 512))
            ext.append(bw(b))
        t["used"] = 2
        pairs.append((t, pair, ext))
        i += 2
    rem = []
    for b in narrow:
        placed = False
        for t, pair, ext in pairs:
            for k in sorted(range(len(pair)), key=lambda k: ext[k]):
                if ext[k] + bw(b) <= 512:
                    t["placements"].append((b, k * 512 + ext[k]))
                    ext[k] += bw(b)
                    placed = True
                    break
            if placed:
                break
        if not placed:
            rem.append(b)
    narrow = rem
    for t, pair, ext in pairs:
        if len(pair) == 2:
            t["exps"].append(("strided", pair, 0, 512, max(ext)))
        else:
            t["exps"].append(("run", pair, 0, ext[0]))

    free_banks = []
    def alloc_bank():
        if not free_banks:
            t = new_tile()
            t["used"] = 2
            free_banks.extend([(t, 0), (t, 512)])
        return free_banks.pop(0)

    if narrow:
        run, runw = [], 0
        bank = alloc_bank()
        for b in narrow:
            if runw + bw(b) > 512:
                t, boff = bank
                t["exps"].append(("run", run, boff, runw))
                bank = alloc_bank()
                run, runw = [], 0
            t, boff = bank
            t["placements"].append((b, boff + runw))
            run.append(b)
            runw += bw(b)
        t, boff = bank
        t["exps"].append(("run", run, boff, runw))
    for b in specials:
        bank = alloc_bank()
        t, boff = bank
        t["placements"].append((b, boff))
        t["exps"].append(("single", b, boff))
    return tiles


# ----------------------------------------------------------------------------
# host-side input prep
# ----------------------------------------------------------------------------
# consts tile layout (fp32, [128, CW]):
#   [0:4)   bq per m-chunk      [4:8) bk
#   [8]     bqml                [9]   bkml
#   [10:20) biascols (exp bias per softmax)
#   [20:30) f_s mix factor per softmax (all partitions; 1 except sm 2,3,8,9)
CONST_BQ, CONST_BK, CONST_BQML, CONST_BKML = 0, 4, 8, 9
CONST_BIAS = 10
CONST_F = 20
CONST_W = 30


def prep_weights(w):
    """Shared (per-batch-invariant) device buffers."""
    f = np.float32
    scale = f(1.0 / np.sqrt(HS))

    wqT = w["w_query"].astype(f).T * scale     # [cin, cout]
    wkT = w["w_key"].astype(f).T
    wvT = w["w_value"].astype(f).T
    wpT = w["w_proj"].astype(f).T
    wqmlT = w["w_query_ml"].astype(f).T * scale  # [512, 128]
    wkmlT = w["w_key_ml"].astype(f).T

    # wqk8: fp8 DoubleRow layout [128, kc(4), 1024] -> [128, 4096]
    # [p, kc, c] = (wq|wk).T[kc*128+p, c] * SW
    wqk = np.ascontiguousarray(
        (np.concatenate([wqT, wkT], axis=1) * SW)
        .reshape(4, 128, 1024).transpose(1, 0, 2).reshape(128, 4096)
    ).astype(F8)
    # wv single tile [128, 4*512]: [p, kc*512+c] = wvT[kc*128+p, c]
    wv = np.ascontiguousarray(
        wvT.reshape(4, 128, 512).transpose(1, 0, 2).reshape(128, 2048)
    ).astype(BF16)
    # wml8 fp8 DR tile [128, 4*256]: per kc [qml 128 | kml 128]
    wml = np.ascontiguousarray(
        (np.concatenate([wqmlT.reshape(4, 128, 128),
                         wkmlT.reshape(4, 128, 128)], axis=2) * SW)
        .transpose(1, 0, 2).reshape(128, 1024)
    ).astype(F8)
    # wp bf16 single tile [128, 4*512] (pairs with bf16 yTn in out-proj)
    wp = np.ascontiguousarray(
        wpT.reshape(4, 128, 512).transpose(1, 0, 2).reshape(128, 2048)
    ).astype(BF16)

    # consts (biascols filled per core)
    consts = np.zeros((128, CONST_W), dtype=f)
    consts[:, CONST_BQ:CONST_BQ + 4] = (w["b_query"].astype(f) * scale
                                        ).reshape(4, 128).T
    consts[:, CONST_BK:CONST_BK + 4] = w["b_key"].astype(f).reshape(4, 128).T
    consts[:, CONST_BQML] = (w["b_query_ml"].astype(f) * scale)
    consts[:, CONST_BKML] = w["b_key_ml"].astype(f)

    wg = w["w_mix"].astype(f)[:, 0, 0, 0]
    wl = w["w_mix"].astype(f)[:, 1, 0, 0]
    fs = np.ones(NSM, dtype=f)
    fs[2], fs[3] = wg[0], wg[1]
    fs[8], fs[9] = wl[0], wl[1]
    consts[:, CONST_F:CONST_F + NSM] = fs[None, :]
    return dict(wqk=wqk, wv=wv, wml=wml, wp=wp, consts=consts)


def core_biascols(w, cond_b):
    f = np.float32
    bias = np.zeros((128, NSM), dtype=f)
    if cond_b > 0:
        clip8 = np.maximum(w["att_bias_clip"].astype(f)[0, :, 0], 0.0) * 10.0
        clip2 = np.maximum(w["att_bias_clip_ml"].astype(f)[0, :, 0], 0.0) * 10.0
        bias[1, :N_HEAD] = clip8
        bias[1, N_HEAD:] = clip2
    return bias


def host_const_shift(w):
    bv = w["b_value"].astype(np.float64)
    wg = w["w_mix"].astype(np.float64)[:, 0, 0, 0]
    wl = w["w_mix"].astype(np.float64)[:, 1, 0, 0]
    scale_h = np.ones(N_HEAD)
    scale_h[2] = wg[0] + wl[0]
    scale_h[3] = wg[1] + wl[1]
    yshift = (bv.reshape(N_HEAD, HS) * scale_h[:, None]).reshape(-1)
    return (yshift @ w["w_proj"].astype(np.float64).T
            + w["b_proj"].astype(np.float64)).astype(np.float32)


# ----------------------------------------------------------------------------
# bass kernel emission
# ----------------------------------------------------------------------------
def emit_kernel(tc, ins, out_ap, plans, mask_offs, mask_w):
    from contextlib import ExitStack
    from concourse import mybir

    nc = tc.nc
    f32 = mybir.dt.float32
    f32r = mybir.dt.float32r
    bf16 = mybir.dt.bfloat16
    AF = mybir.ActivationFunctionType

    def r(ap):
        return ap.bitcast(f32r)

    with ExitStack() as ctx:
        P = ctx.enter_context(tc.tile_pool(name="persist", bufs=1))

        # ---------------- persistent SBUF tiles ----------------
        f8 = mybir.dt.float8e4
        xT = [P.tile([128, T], bf16, name=f"x{k}", tag=f"x{k}") for k in range(4)]
        xt8_sb = P.tile([128, 4 * T], f8, name="xt8", tag="xt8")
        wqk8_sb = P.tile([128, 4096], f8, name="wqk8", tag="wqk8")
        wv_sb = P.tile([128, 2048], bf16, name="wv", tag="wv")
        wml_sb = P.tile([128, 1024], f8, name="wml", tag="wml")
        wp_sb = P.tile([128, 2048], bf16, name="wp", tag="wp")
        consts = P.tile([128, CONST_W], f32, name="consts", tag="consts")
        maskcat = P.tile([128, mask_w], bf16, name="maskcat", tag="maskcat")
        # DoubleRow-ready views [p, kc, cols]
        x8v = xt8_sb[:].rearrange("p (k c) -> p k c", c=T)
        w8v = wqk8_sb[:].rearrange("p (k c) -> p k c", c=1024)
        wml8v = wml_sb[:].rearrange("p (k c) -> p k c", c=256)
        DR = mybir.MatmulPerfMode.DoubleRow

        qT = [P.tile([128, T], bf16, name=f"qT{m}", tag=f"qT{m}") for m in range(4)]
        kT = [P.tile([128, T], bf16, name=f"kT{m}", tag=f"kT{m}") for m in range(4)]
        qml = P.tile([128, T], bf16, name="qml", tag="qml")
        kml = P.tile([128, T], bf16, name="kml", tag="kml")
        vext = [P.tile([128, N_HEAD * 65], bf16, name=f"vx{t}", tag=f"vx{t}")
                for t in range(9)]


        # ---------------- DMA loads ----------------
        # All on the SP queue (HWDGE/DMA-device serialize transfers anyway;
        # keeping ACT's sequencer free for exps). Order = need order.
        nc.sync.dma_start(r(consts[:]), r(ins["consts"][:, :]))
        nc.sync.dma_start(wqk8_sb[:], ins["wqk"][:, :])
        nc.sync.dma_start(xt8_sb[:], ins["xt8"][:, :])
        nc.sync.dma_start(xT[0][:], ins["xt"][0:128, :])
        nc.sync.dma_start(xT[1][:], ins["xt"][128:256, :])
        nc.sync.dma_start(xT[2][:], ins["xt"][256:384, :])
        nc.sync.dma_start(xT[3][:], ins["xt"][384:512, :])
        nc.sync.dma_start(wv_sb[:], ins["wv"][:, :])
        nc.sync.dma_start(wml_sb[:], ins["wml"][:, :])
        nc.sync.dma_start(maskcat[:], ins["masks"][:, :])
        nc.sync.dma_start(wp_sb[:], ins["wp"][:, :])

        # ones columns for the Z row of every AV matmul
        for tt in range(9):
            vx = vext[tt][:].rearrange("p (h e) -> p h e", e=65)
            nc.gpsimd.memset(vx[:, :, 64:65], 1.0)
        # 2x2 identity (tail-transpose operand) rides in the masks buffer
        eye2 = maskcat[0:2, mask_w - 2:mask_w]

        # tile pools (SBUF work tiles)
        ptp = ctx.enter_context(tc.tile_pool(name="ptp", bufs=8))
        ytgp = ctx.enter_context(tc.tile_pool(name="ytgp", bufs=4))   # [128,650] f32
        ynp = ctx.enter_context(tc.tile_pool(name="ynp", bufs=4))     # [128,512] bf16
        mltp = ctx.enter_context(tc.tile_pool(name="mltp", bufs=3))
        rzp = ctx.enter_context(tc.tile_pool(name="rzp", bufs=3))
        ytqp = ctx.enter_context(tc.tile_pool(name="ytqp", bufs=3))

        # psum pools: sp (2 x [128,1024] score tiles = 4 banks) + wp4
        # (4 x [128,512] banks shared by projections, Y^T gen tiles, out-proj
        # and the tail transpose) = 8 banks.
        sp = ctx.enter_context(tc.tile_pool(name="sp", bufs=2, space="PSUM"))
        wp4 = ctx.enter_context(tc.tile_pool(name="wp4", bufs=4, space="PSUM"))

        def alloc_score():
            return sp.tile([128, 1024], f32, name="sp", tag="sp")

        def alloc_small():
            return wp4.tile([128, 512], f32, name="wp4", tag="wp4")

        # ---------------- emission helpers ----------------
        MUL, ADD = mybir.AluOpType.mult, mybir.AluOpType.add

        def evac(dst, ps_ap, bcol):
            """psum -> sbuf bf16 with 1/SW rescale + bias add."""
            nc.vector.tensor_scalar(dst, ps_ap, 1.0 / SW,
                                    consts[:, bcol:bcol + 1],
                                    op0=MUL, op1=ADD)

        def proj_qk1(m, ici, which):
            """q or k projection for head-pair m, query chunk ici (fp8 DR).

            Single-psum so the shared wp4 pool holds at most one projection
            tile at a time alongside the three Y^T gen tiles."""
            i0, W = ICS[ici]
            coff = 0 if which == "q" else 512
            ps = alloc_small()
            for j in range(2):
                nc.tensor.matmul(
                    ps[:, 0:W],
                    lhsT=w8v[:, 2 * j:2 * j + 2,
                             coff + m * 128:coff + (m + 1) * 128],
                    rhs=x8v[:, 2 * j:2 * j + 2, i0:i0 + W],
                    start=(j == 0), stop=(j == 1), perf_mode=DR)
            dst_t = qT if which == "q" else kT
            bcol = (CONST_BQ if which == "q" else CONST_BK) + m
            evac(dst_t[m][:, i0:i0 + W], ps[:, 0:W], bcol)

        def proj_ml1(ici, which):
            i0, W = ICS[ici]
            coff, bcol = ((0, CONST_BQML) if which == "q"
                          else (128, CONST_BKML))
            ps = alloc_small()
            for j in range(2):
                nc.tensor.matmul(
                    ps[:, 0:W],
                    lhsT=wml8v[:, 2 * j:2 * j + 2, coff:coff + 128],
                    rhs=x8v[:, 2 * j:2 * j + 2, i0:i0 + W],
                    start=(j == 0), stop=(j == 1), perf_mode=DR)
            dst = (qml if which == "q" else kml)[:, i0:i0 + W]
            evac(dst, ps[:, 0:W], bcol)

        def proj_v(tt):
            j0, JH = JBS[tt]
            ps = alloc_small()
            for kc in range(4):
                nc.tensor.matmul(
                    ps[0:JH, :],
                    lhsT=xT[kc][:, j0:j0 + JH],
                    rhs=wv_sb[:, kc * 512:(kc + 1) * 512],
                    start=(kc == 0), stop=(kc == 3))
            vx = vext[tt][0:JH].rearrange("p (h e) -> p h e", e=65)
            nc.scalar.activation(
                vx[:, :, 0:64], ps[0:JH, :].rearrange("p (h d) -> p h d", d=64),
                AF.Copy)

        class Chunk:
            """One (softmax, query-chunk): score waves -> per-qgroup AV^T."""

            def __init__(self, s, ici):
                self.s, self.ici = s, ici
                _, self.kind, src_, self.hv = SM_INFO[s]
                self.i0, self.W = ICS[ici]
                if src_ == "main":
                    self.qt, self.kt = qT[s // 2], kT[s // 2]
                    self.off = (s % 2) * 64
                else:
                    self.qt, self.kt, self.off = qml, kml, (s - N_HEAD) * 64
                self.blocks = plans[self.kind][ici]
                self.tiles = build_exp_tiles(self.blocks, self.W)
                self.n_waves = len(self.tiles)
                self.pts = {}

            def score_wave(self, w):
                """One psum tile: its score matmuls, exps, and masks."""
                i0, s = self.i0, self.s
                tile = self.tiles[w]
                st = alloc_score()
                pt = ptp.tile([128, 1024], bf16, name="pt", tag="pt")
                for b, off in tile["placements"]:
                    bwid = b["zhi"] - b["zlo"]
                    nc.tensor.matmul(
                        st[0:b["rows"], off:off + bwid],
                        lhsT=self.kt[self.off:self.off + 64,
                                     b["j0"]:b["j0"] + b["rows"]],
                        rhs=self.qt[self.off:self.off + 64,
                                    i0 + b["zlo"]:i0 + b["zhi"]],
                        start=True, stop=True)
                    self.pts[b["jb"]] = (pt, off, b)
                for exp in tile["exps"]:
                    if exp[0] == "single":
                        _, b, off = exp
                        rows, bwid = b["rows"], b["zhi"] - b["zlo"]
                        if b["bias"]:
                            nc.scalar.activation(
                                pt[0:rows, off:off + bwid],
                                st[0:rows, off:off + bwid], AF.Exp,
                                bias=consts[0:rows,
                                            CONST_BIAS + s:CONST_BIAS + s + 1],
                                scale=1.0)
                        else:
                            nc.scalar.activation(
                                pt[0:rows, off:off + bwid],
                                st[0:rows, off:off + bwid], AF.Exp)
                    elif exp[0] == "run":
                        _, blks, off, wtot = exp
                        nc.scalar.activation(
                            pt[:, off:off + wtot], st[:, off:off + wtot],
                            AF.Exp)
                    else:  # strided
                        _, blks, off0, stride, wmax = exp
                        s0, ng = off0 // stride, len(blks)
                        nc.scalar.activation(
                            pt[:].rearrange("p (g c) -> p g c", c=stride)
                            [:, s0:s0 + ng, 0:wmax],
                            st[:].rearrange("p (g c) -> p g c", c=stride)
                            [:, s0:s0 + ng, 0:wmax],
                            AF.Exp)
                for b, off in tile["placements"]:
                    if b["mask"] is not None:
                        mid, c0, c1 = b["mask"]
                        mo, mw = mask_offs[mid]
                        mask_rr[0] += 1
                        if self.kind == "seq":
                            eng = (nc.gpsimd if mask_rr[0] % 4 == 0
                                   else nc.vector)
                        else:  # alternate loc masks DVE/Pool
                            eng = (nc.gpsimd if mask_rr[0] % 2
                                   else nc.vector)
                        o0 = off + c0 - b["zlo"]
                        eng.tensor_mul(
                            pt[0:b["rows"], o0:o0 + mw],
                            pt[0:b["rows"], o0:o0 + mw],
                            maskcat[0:b["rows"], mo:mo + mw])

            def av_qgroup(self, glo, rows_qg, yt, pos):
                """Accumulate this softmax's AV^T for chunk-relative queries
                [glo, glo+rows_qg) into yt psum cols [pos*65, pos*65+65).

                Output partitions are queries; column 64-of-65 collects the
                softmax denominator via the ones column in vext. blocks[0]
                covers [0, W) so the start=True matmul spans all rows; later
                (partial) blocks always satisfy zlo <= glo (staircase aligns
                with the 128 query grid) and accumulate row subranges."""
                ghi = glo + rows_qg
                blks = [b for b in self.blocks
                        if max(b["zlo"], glo) < min(b["zhi"], ghi)]
                for bi, b in enumerate(blks):
                    assert b["zlo"] <= glo, (self.s, self.ici, glo, b["zlo"])
                    hi = min(b["zhi"], ghi)
                    pt, off, _ = self.pts[b["jb"]]
                    nc.tensor.matmul(
                        yt[0:hi - glo, pos * 65:pos * 65 + 65],
                        lhsT=pt[0:b["rows"],
                                off + glo - b["zlo"]:off + hi - b["zlo"]],
                        rhs=vext[b["jb"]][0:b["rows"],
                                          self.hv * 65:self.hv * 65 + 65],
                        start=(bi == 0), stop=(bi == len(blks) - 1))

        mask_rr = [0]

        # merged output staging: one tile per trio of token chunks
        ost3 = [P.tile([128, 1536], f32, name=f"ost{i}", tag=f"ost{i}")
                for i in range(3)]

        def out_proj(m, ytq, JHt):
            """Out-projection for token chunk m (= query group m).

            ytq: compact transposed tile [128, cc(4), JHt] (c = cc*128+p)."""
            j0, JH = JBS[m]
            yqv = ytq[:].rearrange("p (c t) -> p c t", t=JHt)
            trio, slot = divmod(m, 3)
            po = alloc_small()
            for p in range(4):
                nc.tensor.matmul(
                    po[0:JH, :],
                    lhsT=yqv[:, p, 0:JH],
                    rhs=wp_sb[:, p * 512:(p + 1) * 512],
                    start=(p == 0), stop=(p == 3))
            nc.vector.tensor_copy(ost3[trio][0:JH, slot * 512:slot * 512 + 512],
                                  po[0:JH, :])
            if slot == 2 or m == 8:  # trio complete -> one merged DMA
                t0 = trio * 384
                tw = min(T - t0, 384)
                full = tw // 128
                ov = ost3[trio][:].rearrange("p (s c) -> p s c", c=512)
                nc.sync.dma_start(
                    out_ap[t0:t0 + full * 128, :]
                    .rearrange("(s p) c -> p s c", s=full),
                    ov[:, 0:full])
                if tw % 128:
                    nc.sync.dma_start(
                        out_ap[t0 + full * 128:t0 + tw, :],
                        ost3[trio][0:tw % 128, full * 512:full * 512 + 512])

        def norm_qg(ici, glo, rows_qg, ytg, qg_global):
            """Normalize one query group from its ytg staging and fill yTn.

            rz[:, s] = f_s / Z_s per query partition; y_norm = ytg * rz
            broadcast; ml components scaled by w_l are added into mixed
            heads 2/3; yTn gets the [c, token] layout via DMA transpose
            (PE transpose for the 2-token tail)."""
            ytgv = ytg[0:rows_qg].rearrange("p (s e) -> p s e", e=65)
            rz = rzp.tile([128, 16], f32, name="rz", tag="rz")
            nc.vector.reciprocal(rz[0:rows_qg, 0:NSM], ytgv[:, :, 64])
            nc.vector.tensor_tensor(
                rz[0:rows_qg, 0:NSM], rz[0:rows_qg, 0:NSM],
                consts[0:rows_qg, CONST_F:CONST_F + NSM], op=MUL)
            yn = ynp.tile([128, 512], bf16, name="yn", tag="yn")
            mlt = mltp.tile([128, 128], bf16, name="mlt", tag="mlt")
            ynv = yn[0:rows_qg].rearrange("p (s e) -> p s e", e=64)
            mlv = mlt[0:rows_qg].rearrange("p (s e) -> p s e", e=64)
            nc.vector.tensor_tensor(
                ynv[:, 0:8], ytgv[:, 0:8, 0:64],
                rz[0:rows_qg, 0:8, None].broadcast_to((rows_qg, 8, 64)),
                op=MUL)
            nc.vector.tensor_tensor(
                mlv[:, 0:2], ytgv[:, 8:10, 0:64],
                rz[0:rows_qg, 8:10, None].broadcast_to((rows_qg, 2, 64)),
                op=MUL)
            nc.vector.tensor_tensor(yn[0:rows_qg, 128:192],
                                    yn[0:rows_qg, 128:192],
                                    mlt[0:rows_qg, 0:64], op=ADD)
            nc.vector.tensor_tensor(yn[0:rows_qg, 192:256],
                                    yn[0:rows_qg, 192:256],
                                    mlt[0:rows_qg, 64:128], op=ADD)
            if DEBUG_TAPS and qg_global == DEBUG_QG:
                dbg_ytg = P.tile([128, 650], f32, name="dytg", tag="dytg")
                dbg_yn = P.tile([128, 512], bf16, name="dyn", tag="dyn")
                nc.vector.tensor_copy(dbg_ytg[0:rows_qg, :], ytg[0:rows_qg, :])
                nc.vector.tensor_copy(dbg_yn[0:rows_qg, :], yn[0:rows_qg, :])
                for nm, t in (("ytgq", dbg_ytg), ("ynq", dbg_yn)):
                    dst = nc.dram_tensor(f"dbg_{nm}", [128, t.shape[1]],
                                         t[:].dtype, kind="ExternalOutput").ap()
                    nc.sync.dma_start(dst[:, :], t[:])
            if rows_qg >= 16:
                # one transpose DMA -> compact [128, 4, rows] tile
                # (out[p, cc, t] = yn[t, cc*128+p]; out must be contiguous)
                ytq = ytqp.tile([128, 512], bf16, name="ytq", tag="ytq")
                nc.sync.dma_start_transpose(
                    ytq[:].rearrange("p (c t) -> p c t", t=rows_qg),
                    yn[0:rows_qg, :])
                out_proj(qg_global, ytq, rows_qg)
            else:  # 2-token tail: PE transpose through a bf16 psum tile
                tps = wp4.tile([128, 1024], bf16, name="tp", tag="wp4")
                for cc in range(4):
                    nc.tensor.transpose(
                        tps[:, cc * 2:cc * 2 + 2],
                        yn[0:rows_qg, cc * 128:(cc + 1) * 128], eye2)
                ytq = ytqp.tile([128, 512], bf16, name="ytq", tag="ytq")
                nc.vector.tensor_copy(ytq[:, 0:4 * rows_qg], tps[:, 0:8])
                out_proj(qg_global, ytq, rows_qg)

        def process_chunk(ici, fillers):
            """All 10 softmaxes of one query chunk, in two 5-softmax gens.

            Per softmax: score waves -> (fillers) -> previous softmax's AV^T
            (one softmax behind, hiding exp/mask latency). Gen g's Y^T psum
            tiles (one bank per qgroup) evacuate into ytg[:, g*325:...] when
            the gen's last softmax has AV'd; after gen 1, each qgroup is
            normalized, transposed into yTn, and its out-projection emitted."""
            qgs = chunk_qgroups(ici)
            base_qg = sum(len(chunk_qgroups(i)) for i in range(ici))
            ytg_t = [ytgp.tile([128, 650], f32, name="ytg", tag="ytg")
                     for _ in qgs]
            gen_tiles = {}
            pend = None

            def flush(pend_ch):
                ch, gi = pend_ch
                if gi not in gen_tiles:
                    gen_tiles[gi] = [
                        wp4.tile([128, 512], f32, name="yt", tag="wp4")
                        for _ in qgs]
                for qi, (glo, rows_qg) in enumerate(qgs):
                    ch.av_qgroup(glo, rows_qg, gen_tiles[gi][qi],
                                 POS[ch.s] % 5)
                if ch.s == GEN_SMS[gi][-1]:  # gen complete -> evacuate
                    for qi, (glo, rows_qg) in enumerate(qgs):
                        nc.vector.tensor_copy(
                            ytg_t[qi][0:rows_qg, gi * 325:gi * 325 + 325],
                            gen_tiles[gi][qi][0:rows_qg, 0:325])

            for gi, sms in enumerate(GEN_SMS):
                for s in sms:
                    si = POS[s]
                    ch = Chunk(s, ici)
                    for w in range(ch.n_waves):
                        ch.score_wave(w)
                    for f in fillers.get(si, []):
                        f()
                    if pend is not None:
                        flush(pend)
                    pend = (ch, gi)
            flush(pend)
            # defer norms/out-projs into the next chunk's slots so their
            # psum/pool allocations trail the next chunk's gen tiles
            return [(lambda glo=glo, rows_qg=rows_qg, t=t, q=q:
                     norm_qg(ici, glo, rows_qg, t, q))
                    for (glo, rows_qg), t, q in
                    zip(qgs, ytg_t, range(base_qg, base_qg + len(qgs)))]

        # ---------------- emission schedule ----------------
        # Chunk-major. Projections for chunk ici+1 ride as fillers inside
        # chunk ici; all of chunk 0's own projections are emitted up front /
        # in its first softmax slots (DMA-gated anyway).
        fillers0 = {
            0: [lambda: proj_v(0), lambda: proj_v(1),
                lambda: proj_qk1(1, 0, "q"), lambda: proj_qk1(1, 0, "k")],
            1: [lambda: proj_v(2), lambda: proj_v(3),
                lambda: proj_qk1(2, 0, "q"), lambda: proj_qk1(2, 0, "k")],
            2: [lambda: proj_qk1(3, 0, "q"), lambda: proj_qk1(3, 0, "k"),
                lambda: proj_v(4)],
            3: [lambda: proj_ml1(0, "q"), lambda: proj_ml1(0, "k"),
                lambda: proj_v(5)],
            4: [lambda: proj_qk1(0, 1, "q"), lambda: proj_qk1(0, 1, "k"),
                lambda: proj_v(6)],
            5: [lambda: proj_qk1(1, 1, "q"), lambda: proj_qk1(1, 1, "k"),
                lambda: proj_v(7)],
            6: [lambda: proj_qk1(2, 1, "q"), lambda: proj_qk1(2, 1, "k"),
                lambda: proj_v(8)],
            7: [lambda: proj_qk1(3, 1, "q"), lambda: proj_qk1(3, 1, "k")],
            8: [lambda: proj_ml1(1, "q"), lambda: proj_ml1(1, "k")],
            9: [lambda: proj_qk1(0, 2, "q"), lambda: proj_qk1(0, 2, "k")],
        }
        fillers1 = {
            0: [lambda: proj_qk1(1, 2, "q"), lambda: proj_qk1(1, 2, "k")],
            1: [lambda: proj_qk1(2, 2, "q"), lambda: proj_qk1(2, 2, "k")],
            2: [lambda: proj_qk1(3, 2, "q"), lambda: proj_qk1(3, 2, "k")],
            3: [lambda: proj_ml1(2, "q"), lambda: proj_ml1(2, "k")],
        }
        proj_qk1(0, 0, "q")
        proj_qk1(0, 0, "k")
        d0 = process_chunk(0, fillers0)
        for si, d in zip((2, 3, 4), d0):
            fillers1.setdefault(si, []).append(d)
        d1 = process_chunk(1, fillers1)
        fillers2 = {si: [d] for si, d in zip((2, 3, 4), d1)}
        d2 = process_chunk(2, fillers2)
        for d in d2:
            d()

        if DEBUG_TAPS:
            taps = dict(qT0=qT[0], kT0=kT[0], qml=qml, vx0=vext[0],
                        yTn4=yTn4, xt8=xt8_sb)
            for nm in DEBUG_TAPS:
                t = taps[nm]
                shp = [t.shape[0], t.shape[1]]
                dt_ = t[:].dtype
                dst = nc.dram_tensor(f"dbg_{nm}", shp, dt_,
                                     kind="ExternalOutput").ap()
                nc.sync.dma_start(dst[:, :], t[:])


# ----------------------------------------------------------------------------
# module build + run
# ----------------------------------------------------------------------------
_CACHE = {}


def _get_module():
    if "nc" in _CACHE:
        return _CACHE["nc"], _CACHE["plans"], _CACHE["mask_offs"], _CACHE["maskcat"]
    import concourse.tile as tile
    from concourse import bacc, mybir

    plans, maskcat, mask_offs = build_block_plan()
    eye = np.zeros((128, 2), np.float32)
    eye[0, 0] = eye[1, 1] = 1.0
    maskcat = (np.concatenate([maskcat, eye], axis=1)
               if maskcat.shape[1] else eye)
    mask_w = maskcat.shape[1]

    nc = bacc.Bacc("TRN2", target_bir_lowering=False, debug=False,
                   enable_asserts=False, num_devices=NCORES)
    f32 = mybir.dt.float32
    bf16 = mybir.dt.bfloat16
    f8 = mybir.dt.float8e4

    def din(name, shape, dt=f32):
        return nc.dram_tensor(name, list(shape), dt, kind="ExternalInput").ap()

    ins = dict(
        xt=din("xt", (EMBED, T), bf16),
        xt8=din("xt8", (128, 4 * T), f8),
        wqk=din("wqk", (128, 4096), f8),
        wv=din("wv", (128, 2048), bf16),
        wml=din("wml", (128, 1024), f8),
        wp=din("wp", (128, 2048), f32),
        consts=din("consts", (128, CONST_W), f32),
        masks=din("masks", (128, mask_w), bf16),
    )
    out_ap = nc.dram_tensor("out_p", [T, EMBED], f32, kind="ExternalOutput").ap()

    with tile.TileContext(nc) as tc:
        emit_kernel(tc, ins, out_ap, plans, mask_offs, mask_w)
    nc.compile()

    _CACHE.update(nc=nc, plans=plans, mask_offs=mask_offs, maskcat=maskcat)
    return nc, plans, mask_offs, maskcat


def build_in_maps(inputs):
    nc, plans, mask_offs, maskcat = _get_module()
    x = inputs["x"].astype(np.float32)
    cond = np.asarray(inputs["cond_mask"]).astype(np.int32)
    B = x.shape[0]
    assert B == NCORES, f"expected B={NCORES}, got {B}"

    ws = prep_weights(inputs)  # weights may differ between calls
    if "masks_bf" not in _CACHE:  # masks are static problem constants
        mc = maskcat if maskcat.shape[1] else np.zeros((128, 2), np.float32)
        _CACHE["masks_bf"] = mc.astype(BF16)
    perm, _ = build_perm()

    in_maps = []
    bias_cache = {}
    for b in range(B):
        cb = int(cond[b])
        if cb not in bias_cache:
            consts = ws["consts"].copy()
            consts[:, CONST_BIAS:CONST_BIAS + NSM] = core_biascols(inputs, cb)
            bias_cache[cb] = consts
        xtb = np.ascontiguousarray(x[b][perm].T)  # [512, T]
        xt8 = np.ascontiguousarray(
            xtb.reshape(4, 128, T).transpose(1, 0, 2).reshape(128, 4 * T)
        ).astype(F8)
        in_maps.append(dict(
            xt=xtb.astype(BF16), xt8=xt8,
            wqk=ws["wqk"], wv=ws["wv"], wml=ws["wml"], wp=ws["wp"],
            consts=bias_cache[cb], masks=_CACHE["masks_bf"],
        ))
    return nc, in_maps


def kernel(**inputs):
    from concourse import bass_utils

    inputs = {k: np.asarray(v) for k, v in inputs.items()}
    nc, in_maps = build_in_maps(inputs)
    res = bass_utils.run_bass_kernel_spmd(nc, in_maps, core_ids=list(range(NCORES)))
    _CACHE["last_results"] = res

    _, inv = build_perm()
    shift = host_const_shift(inputs)
    B = inputs["x"].shape[0]
    out = np.empty((B, T, EMBED), dtype=np.float32)
    for b in range(B):
        out[b] = res.results[b]["out_p"][inv] + shift
    return out



# revision 55
# speedup vs baseline: 1.0129x; 1.0129x over previous
"""Trainium2 Bass kernel for nn_CausalCrossConditionalSelfAttention.

Data-parallel over batch B=8, one element per core. Design:
  - Exact T=1026 (no padding): query chunks (384,386,256), key blocks
    8x128 + one 2-row tiny block; scores/exp/AV operate on per-block
    column ranges [zlo,zhi) so the causal triangle / local band is not
    padded to full chunk width.
  - bf16 for x/weights/qT/kT/pt/vext/masks (halves DMA, 2x DVE mask-muls,
    any-N matmuls); fp32 psums, Z path, and final out-projection.
  - ~15 DMAs total (each DMACopy costs ~630ns on the shared HWDGE).
  - Softmax denominators ride as a ones-column in the AV matmul; Z rows are
    scaled by 1/mix-factor into a [65,W] staging tile (partitions 0/64),
    broadcast to 128 partitions by one select-matmul per head-pair, and
    applied via reciprocal + in-place multiply (mix weights folded in).
  - Score blocks bin-packed into [128,1024] psum tiles (256-wide slots for
    the 256 chunk, contiguous runs for narrow local-band blocks, stride-512
    pairs for wide blocks) to minimize exp instruction count.
  - One global software-pipelined emission pass: chunk n's score waves
    interleave with chunk n-2's AV matmuls; projections and out-projections
    fill PE slack; per-head-pair normalization fires as soon as both
    members finish.
"""

import sys

if "/opt/trn_rl_repo" not in sys.path:
    sys.path.insert(0, "/opt/trn_rl_repo")

import numpy as np

try:
    import ml_dtypes
    BF16 = np.dtype(ml_dtypes.bfloat16)
    F8 = np.dtype(ml_dtypes.float8_e4m3)
except ImportError:  # pragma: no cover
    BF16 = None
    F8 = None

# fp8 weight scale: w*scale values (~0.0025) sit in e4m3's subnormal range,
# so store w*SW and multiply psum by 1/SW in the evacuation op.
SW = 256.0
DEBUG_TAPS = ()
DEBUG_QG = 1

# ----------------------------------------------------------------------------
# problem constants
# ----------------------------------------------------------------------------
BLOCK = 512
RECEP = 4
N_HEAD = 8
EMBED = 512
HS = 64
T = 2 * BLOCK + 2          # 1026
NSM = 10
NCORES = 8

# query chunks (offset, width): 128-aligned starts so the causal staircase's
# block zlo values never land inside a 128-query group (AV out base always 0)
ICS = [(0, 384), (384, 384), (768, 258)]
# key blocks (offset, height)
JBS = [(j * 128, 128) for j in range(8)] + [(1024, 2)]

# softmax id -> (mask kind, q/k source, v head)
SM_INFO = [
    (0, "loc", "main", 0), (1, "loc", "main", 1),
    (2, "seq", "main", 2), (3, "seq", "main", 3),
    (4, "seq", "main", 4), (5, "seq", "main", 5),
    (6, "seq", "main", 6), (7, "seq", "main", 7),
    (8, "loc", "ml", 2), (9, "loc", "ml", 3),
]
# softmax emission generations per chunk: Y^T psum gen tiles hold 5 softmaxes
# (5*65=325 cols, one bank); ytg staging column position of softmax s
GEN_SMS = [[0, 1, 2, 3, 4], [5, 6, 7, 8, 9]]
POS = {s: gi * 5 + i for gi, g in enumerate(GEN_SMS) for i, s in enumerate(g)}


def chunk_qgroups(ici):
    i0, W = ICS[ici]
    return [(g * 128, min(128, W - g * 128)) for g in range((W + 127) // 128)]


# ----------------------------------------------------------------------------
# host-side plan construction
# ----------------------------------------------------------------------------
def build_perm():
    perm = np.zeros(T, dtype=np.int64)
    perm[0], perm[1] = 0, 1
    b = np.arange(BLOCK)
    perm[2 + 2 * b] = 2 + b
    perm[3 + 2 * b] = 2 + BLOCK + b
    inv = np.argsort(perm)
    return perm, inv


def build_masks_orig():
    to = np.concatenate([np.zeros(2), np.arange(BLOCK) * 2 + 1, np.arange(BLOCK) * 2 + 2])
    seq = to[None, :] <= to[:, None]
    qo = np.concatenate([np.arange(BLOCK) * 2 + 1 - 2 * RECEP + 1] * 2)
    ko = np.concatenate([np.arange(BLOCK) * 2 + 1] * 2)
    de = ko[None, :] < qo[:, None]
    loc = seq.copy()
    loc[2:, 2:] = loc[2:, 2:] & (~de)
    return seq, loc


def build_block_plan():
    """Per (kind, ic): list of block dicts with exact column ranges.

    block = dict(jb, j0, rows, zlo, zhi, bias, mask=(mid,c0,c1) or None)
    Ordered so the first block covers [0, W) (widest) for PSUM start=True.
    """
    perm, _ = build_perm()
    seq, loc = build_masks_orig()
    Ms = seq[perm][:, perm]
    Ml = loc[perm][:, perm]

    mask_tiles = []
    tile_index = {}

    def tile_id(tile):
        key = tile.tobytes() + bytes(str(tile.shape), "ascii")
        if key not in tile_index:
            tile_index[key] = len(mask_tiles)
            mask_tiles.append(tile)
        return tile_index[key]

    plans = {}
    for kind, M in (("seq", Ms), ("loc", Ml)):
        plan = []
        for i0, W in ICS:
            blocks = []
            for jb, (j0, JH) in enumerate(JBS):
                sub = M[i0:i0 + W, j0:j0 + JH].T  # [JH, W] keys x queries
                if not sub.any():
                    continue
                nz_rows = np.flatnonzero(sub.any(axis=1))
                rows = int(nz_rows.max()) + 1
                colmask = sub[:rows].any(axis=0)
                nz_cols = np.flatnonzero(colmask)
                zlo, zhi = int(nz_cols.min()), int(nz_cols.max()) + 1
                core = sub[:rows, zlo:zhi]
                if core.all():
                    mask = None
                else:
                    pc = np.flatnonzero(~core.all(axis=0))
                    c0, c1 = zlo + int(pc.min()), zlo + int(pc.max()) + 1
                    mid = tile_id(
                        sub[:rows, c0:c1].astype(np.float32).copy())
                    mask = (mid, c0, c1)
                blocks.append(dict(jb=jb, j0=j0, rows=rows, zlo=zlo, zhi=zhi,
                                   bias=(j0 == 0), mask=mask))
            # widest-coverage block first (needed for PSUM start=True)
            blocks.sort(key=lambda b: (b["zlo"], -b["zhi"]))
            assert blocks[0]["zlo"] == 0 and blocks[0]["zhi"] == W, (kind, i0)
            plan.append(blocks)
        plans[kind] = plan

    offs, cat = [], []
    o = 0
    for t in mask_tiles:
        offs.append((o, t.shape[1]))
        cat.append(np.pad(t, ((0, 128 - t.shape[0]), (0, 0))))
        o += t.shape[1]
    maskcat = (np.concatenate(cat, axis=1) if cat
               else np.zeros((128, 0), np.float32))
    return plans, maskcat, offs


def build_exp_tiles(blocks, W):
    """Pack a chunk's blocks into [128,1024] score-psum tiles.

    Returns a list of tiles; each tile is a dict:
      placements: [(block, off)]          off in [0,1024), bank-contained
      exps: [("single", block, off)]      bias / tiny blocks
            [("run", [blocks], off, w)]   contiguous narrow blocks, one bank
            [("strided", [blocks], off0, stride, wmax)]
    """
    def bw(b):
        return b["zhi"] - b["zlo"]

    specials = [b for b in blocks if b["bias"] or b["rows"] < 128]
    plain = sorted((b for b in blocks if not (b["bias"] or b["rows"] < 128)),
                   key=lambda b: b["jb"])
    tiles = []

    def new_tile():
        tiles.append(dict(placements=[], exps=[], used=0))
        return tiles[-1]

    if W <= 256:
        # uniform 256-wide slots, 4 per tile; strided exps over plain runs
        slots = specials + plain  # bias first, then jb order
        t = None
        for i, b in enumerate(slots):
            si = i % 4
            if si == 0:
                t = new_tile()
            t["placements"].append((b, si * 256))
        # exps: walk slots; specials single, plain grouped per tile
        for ti, t in enumerate(tiles):
            runb, ro, wmax = [], 0, 0
            for b, off in t["placements"]:
                if b["bias"] or b["rows"] < 128:
                    t["exps"].append(("single", b, off))
                else:
                    if not runb:
                        ro = off
                    runb.append(b)
                    wmax = max(wmax, bw(b))
            if runb:
                t["exps"].append(("strided", runb, ro, 256, wmax))
        return tiles

    wide = [b for b in plain if bw(b) > 256]
    narrow = [b for b in plain if bw(b) <= 256]
    # wide: stride-512 pairs occupying a full tile; narrow leftovers are
    # appended into pair spare bank space, extending the exp width (the
    # shorter bank's tail exps stale psum, which is never read downstream)
    pairs = []
    i = 0
    while i < len(wide):
        t = new_tile()
        pair = wide[i:i + 2]
        ext = []
        for g, b in enumerate(pair):
            t["placements"].append((b, g * 512))
            ext.append(bw(b))
        t["used"] = 2
        pairs.append((t, pair, ext))
        i += 2
    rem = []
    for b in narrow:
        placed = False
        for t, pair, ext in pairs:
            for k in sorted(range(len(pair)), key=lambda k: ext[k]):
                if ext[k] + bw(b) <= 512:
                    t["placements"].append((b, k * 512 + ext[k]))
                    ext[k] += bw(b)
                    placed = True
                    break
            if placed:
                break
        if not placed:
            rem.append(b)
    narrow = rem
    for t, pair, ext in pairs:
        if len(pair) == 2:
            t["exps"].append(("strided", pair, 0, 512, max(ext)))
        else:
            t["exps"].append(("run", pair, 0, ext[0]))

    free_banks = []
    def alloc_bank():
        if not free_banks:
            t = new_tile()
            t["used"] = 2
            free_banks.extend([(t, 0), (t, 512)])
        return free_banks.pop(0)

    if narrow:
        run, runw = [], 0
        bank = alloc_bank()
        for b in narrow:
            if runw + bw(b) > 512:
                t, boff = bank
                t["exps"].append(("run", run, boff, runw))
                bank = alloc_bank()
                run, runw = [], 0
            t, boff = bank
            t["placements"].append((b, boff + runw))
            run.append(b)
            runw += bw(b)
        t, boff = bank
        t["exps"].append(("run", run, boff, runw))
    for b in specials:
        bank = alloc_bank()
        t, boff = bank
        t["placements"].append((b, boff))
        t["exps"].append(("single", b, boff))
    return tiles


# ----------------------------------------------------------------------------
# host-side input prep
# ----------------------------------------------------------------------------
# consts tile layout (fp32, [128, CW]):
#   [0:4)   bq per m-chunk      [4:8) bk
#   [8]     bqml                [9]   bkml
#   [10:20) biascols (exp bias per softmax)
#   [20:30) f_s mix factor per softmax (all partitions; 1 except sm 2,3,8,9)
CONST_BQ, CONST_BK, CONST_BQML, CONST_BKML = 0, 4, 8, 9
CONST_BIAS = 10
CONST_F = 20
CONST_W = 30


def prep_weights(w):
    """Shared (per-batch-invariant) device buffers."""
    f = np.float32
    scale = f(1.0 / np.sqrt(HS))

    wqT = w["w_query"].astype(f).T * scale     # [cin, cout]
    wkT = w["w_key"].astype(f).T
    wvT = w["w_value"].astype(f).T
    wpT = w["w_proj"].astype(f).T
    wqmlT = w["w_query_ml"].astype(f).T * scale  # [512, 128]
    wkmlT = w["w_key_ml"].astype(f).T

    # wqk8: fp8 DoubleRow layout [128, kc(4), 1024] -> [128, 4096]
    # [p, kc, c] = (wq|wk).T[kc*128+p, c] * SW
    wqk = np.ascontiguousarray(
        (np.concatenate([wqT, wkT], axis=1) * SW)
        .reshape(4, 128, 1024).transpose(1, 0, 2).reshape(128, 4096)
    ).astype(F8)
    # wv single tile [128, 4*512]: [p, kc*512+c] = wvT[kc*128+p, c]
    wv = np.ascontiguousarray(
        wvT.reshape(4, 128, 512).transpose(1, 0, 2).reshape(128, 2048)
    ).astype(BF16)
    # wml8 fp8 DR tile [128, 4*256]: per kc [qml 128 | kml 128]
    wml = np.ascontiguousarray(
        (np.concatenate([wqmlT.reshape(4, 128, 128),
                         wkmlT.reshape(4, 128, 128)], axis=2) * SW)
        .transpose(1, 0, 2).reshape(128, 1024)
    ).astype(F8)
    # wp bf16 single tile [128, 4*512] (pairs with bf16 yTn in out-proj)
    wp = np.ascontiguousarray(
        wpT.reshape(4, 128, 512).transpose(1, 0, 2).reshape(128, 2048)
    ).astype(BF16)

    # consts (biascols filled per core)
    consts = np.zeros((128, CONST_W), dtype=f)
    consts[:, CONST_BQ:CONST_BQ + 4] = (w["b_query"].astype(f) * scale
                                        ).reshape(4, 128).T
    consts[:, CONST_BK:CONST_BK + 4] = w["b_key"].astype(f).reshape(4, 128).T
    consts[:, CONST_BQML] = (w["b_query_ml"].astype(f) * scale)
    consts[:, CONST_BKML] = w["b_key_ml"].astype(f)

    wg = w["w_mix"].astype(f)[:, 0, 0, 0]
    wl = w["w_mix"].astype(f)[:, 1, 0, 0]
    fs = np.ones(NSM, dtype=f)
    fs[2], fs[3] = wg[0], wg[1]
    fs[8], fs[9] = wl[0], wl[1]
    consts[:, CONST_F:CONST_F + NSM] = fs[None, :]
    return dict(wqk=wqk, wv=wv, wml=wml, wp=wp, consts=consts)


def core_biascols(w, cond_b):
    f = np.float32
    bias = np.zeros((128, NSM), dtype=f)
    if cond_b > 0:
        clip8 = np.maximum(w["att_bias_clip"].astype(f)[0, :, 0], 0.0) * 10.0
        clip2 = np.maximum(w["att_bias_clip_ml"].astype(f)[0, :, 0], 0.0) * 10.0
        bias[1, :N_HEAD] = clip8
        bias[1, N_HEAD:] = clip2
    return bias


def host_const_shift(w):
    bv = w["b_value"].astype(np.float64)
    wg = w["w_mix"].astype(np.float64)[:, 0, 0, 0]
    wl = w["w_mix"].astype(np.float64)[:, 1, 0, 0]
    scale_h = np.ones(N_HEAD)
    scale_h[2] = wg[0] + wl[0]
    scale_h[3] = wg[1] + wl[1]
    yshift = (bv.reshape(N_HEAD, HS) * scale_h[:, None]).reshape(-1)
    return (yshift @ w["w_proj"].astype(np.float64).T
            + w["b_proj"].astype(np.float64)).astype(np.float32)


# ----------------------------------------------------------------------------
# bass kernel emission
# ----------------------------------------------------------------------------
def emit_kernel(tc, ins, out_ap, plans, mask_offs, mask_w):
    from contextlib import ExitStack
    from concourse import mybir

    nc = tc.nc
    f32 = mybir.dt.float32
    f32r = mybir.dt.float32r
    bf16 = mybir.dt.bfloat16
    AF = mybir.ActivationFunctionType

    def r(ap):
        return ap.bitcast(f32r)

    with ExitStack() as ctx:
        P = ctx.enter_context(tc.tile_pool(name="persist", bufs=1))

        # ---------------- persistent SBUF tiles ----------------
        f8 = mybir.dt.float8e4
        xT = [P.tile([128, T], bf16, name=f"x{k}", tag=f"x{k}") for k in range(4)]
        xt8_sb = P.tile([128, 4 * T], f8, name="xt8", tag="xt8")
        wqk8_sb = P.tile([128, 4096], f8, name="wqk8", tag="wqk8")
        wv_sb = P.tile([128, 2048], bf16, name="wv", tag="wv")
        wml_sb = P.tile([128, 1024], f8, name="wml", tag="wml")
        wp_sb = P.tile([128, 2048], bf16, name="wp", tag="wp")
        consts = P.tile([128, CONST_W], f32, name="consts", tag="consts")
        maskcat = P.tile([128, mask_w], bf16, name="maskcat", tag="maskcat")
        # DoubleRow-ready views [p, kc, cols]
        x8v = xt8_sb[:].rearrange("p (k c) -> p k c", c=T)
        w8v = wqk8_sb[:].rearrange("p (k c) -> p k c", c=1024)
        wml8v = wml_sb[:].rearrange("p (k c) -> p k c", c=256)
        DR = mybir.MatmulPerfMode.DoubleRow

        qT = [P.tile([128, T], bf16, name=f"qT{m}", tag=f"qT{m}") for m in range(4)]
        kT = [P.tile([128, T], bf16, name=f"kT{m}", tag=f"kT{m}") for m in range(4)]
        qml = P.tile([128, T], bf16, name="qml", tag="qml")
        kml = P.tile([128, T], bf16, name="kml", tag="kml")
        vext = [P.tile([128, N_HEAD * 65], bf16, name=f"vx{t}", tag=f"vx{t}")
                for t in range(9)]


        # ---------------- DMA loads ----------------
        # All on the SP queue (HWDGE/DMA-device serialize transfers anyway;
        # keeping ACT's sequencer free for exps). Order = need order.
        nc.sync.dma_start(r(consts[:]), r(ins["consts"][:, :]))
        nc.sync.dma_start(wqk8_sb[:], ins["wqk"][:, :])
        nc.sync.dma_start(xt8_sb[:], ins["xt8"][:, :])
        nc.sync.dma_start(maskcat[:], ins["masks"][:, :])
        nc.sync.dma_start(xT[0][:], ins["xt"][0:128, :])
        nc.sync.dma_start(xT[1][:], ins["xt"][128:256, :])
        nc.sync.dma_start(wv_sb[:], ins["wv"][:, :])
        nc.sync.dma_start(xT[2][:], ins["xt"][256:384, :])
        nc.sync.dma_start(xT[3][:], ins["xt"][384:512, :])
        nc.sync.dma_start(wml_sb[:], ins["wml"][:, :])
        nc.sync.dma_start(wp_sb[:], ins["wp"][:, :])

        # ones columns for the Z row of every AV matmul
        for tt in range(9):
            vx = vext[tt][:].rearrange("p (h e) -> p h e", e=65)
            nc.gpsimd.memset(vx[:, :, 64:65], 1.0)
        # 2x2 identity (tail-transpose operand) rides in the masks buffer
        eye2 = maskcat[0:2, mask_w - 2:mask_w]
        # bf16 copy of the mix factors (pairs with bf16 rz in the norm)
        fcol_bf = P.tile([128, 16], bf16, name="fcol", tag="fcol")
        nc.vector.tensor_copy(fcol_bf[:, 0:NSM],
                              consts[:, CONST_F:CONST_F + NSM])

        # tile pools (SBUF work tiles)
        ptp = ctx.enter_context(tc.tile_pool(name="ptp", bufs=8))
        ytgp = ctx.enter_context(tc.tile_pool(name="ytgp", bufs=4))   # [128,650] f32
        ynp = ctx.enter_context(tc.tile_pool(name="ynp", bufs=4))     # [128,512] bf16
        mltp = ctx.enter_context(tc.tile_pool(name="mltp", bufs=3))
        rzp = ctx.enter_context(tc.tile_pool(name="rzp", bufs=3))
        ytqp = ctx.enter_context(tc.tile_pool(name="ytqp", bufs=3))

        # psum pools: sp (2 x [128,1024] score tiles = 4 banks) + genp
        # (3 x 1-bank long-lived Y^T gen tiles) + smallp (1 bank rotating
        # through transient projection/out-proj/tail tiles) = 8 banks.
        # Long-lived and transient tiles MUST NOT share a pool: rotation
        # could hand a transient a buffer owned by a live gen tile, putting
        # a PE instruction ahead of the AVs that free it (deadlock).
        sp = ctx.enter_context(tc.tile_pool(name="sp", bufs=2, space="PSUM"))
        wp4 = ctx.enter_context(tc.tile_pool(name="wp4", bufs=4, space="PSUM"))
        genp = wp4
        smallp = wp4

        def alloc_score():
            return sp.tile([128, 1024], f32, name="sp", tag="sp")

        def alloc_small():
            return smallp.tile([128, 512], f32, name="smallp", tag="wp4")

        # ---------------- emission helpers ----------------
        MUL, ADD = mybir.AluOpType.mult, mybir.AluOpType.add

        def evac(dst, ps_ap, bcol):
            """psum -> sbuf bf16 with 1/SW rescale + bias add."""
            nc.vector.tensor_scalar(dst, ps_ap, 1.0 / SW,
                                    consts[:, bcol:bcol + 1],
                                    op0=MUL, op1=ADD)

        def proj_qk1(m, ici, which):
            """q or k projection for head-pair m, query chunk ici (fp8 DR).

            Single-psum so the shared wp4 pool holds at most one projection
            tile at a time alongside the three Y^T gen tiles."""
            i0, W = ICS[ici]
            coff = 0 if which == "q" else 512
            ps = alloc_small()
            for j in range(2):
                nc.tensor.matmul(
                    ps[:, 0:W],
                    lhsT=w8v[:, 2 * j:2 * j + 2,
                             coff + m * 128:coff + (m + 1) * 128],
                    rhs=x8v[:, 2 * j:2 * j + 2, i0:i0 + W],
                    start=(j == 0), stop=(j == 1), perf_mode=DR)
            dst_t = qT if which == "q" else kT
            bcol = (CONST_BQ if which == "q" else CONST_BK) + m
            evac(dst_t[m][:, i0:i0 + W], ps[:, 0:W], bcol)

        def proj_ml1(ici, which):
            i0, W = ICS[ici]
            coff, bcol = ((0, CONST_BQML) if which == "q"
                          else (128, CONST_BKML))
            ps = alloc_small()
            for j in range(2):
                nc.tensor.matmul(
                    ps[:, 0:W],
                    lhsT=wml8v[:, 2 * j:2 * j + 2, coff:coff + 128],
                    rhs=x8v[:, 2 * j:2 * j + 2, i0:i0 + W],
                    start=(j == 0), stop=(j == 1), perf_mode=DR)
            dst = (qml if which == "q" else kml)[:, i0:i0 + W]
            evac(dst, ps[:, 0:W], bcol)

        def proj_v(tt):
            j0, JH = JBS[tt]
            ps = alloc_small()
            for kc in range(4):
                nc.tensor.matmul(
                    ps[0:JH, :],
                    lhsT=xT[kc][:, j0:j0 + JH],
                    rhs=wv_sb[:, kc * 512:(kc + 1) * 512],
                    start=(kc == 0), stop=(kc == 3))
            vx = vext[tt][0:JH].rearrange("p (h e) -> p h e", e=65)
            nc.scalar.activation(
                vx[:, :, 0:64], ps[0:JH, :].rearrange("p (h d) -> p h d", d=64),
                AF.Copy)

        class Chunk:
            """One (softmax, query-chunk): score waves -> per-qgroup AV^T."""

            def __init__(self, s, ici):
                self.s, self.ici = s, ici
                _, self.kind, src_, self.hv = SM_INFO[s]
                self.i0, self.W = ICS[ici]
                if src_ == "main":
                    self.qt, self.kt = qT[s // 2], kT[s // 2]
                    self.off = (s % 2) * 64
                else:
                    self.qt, self.kt, self.off = qml, kml, (s - N_HEAD) * 64
                self.blocks = plans[self.kind][ici]
                self.tiles = build_exp_tiles(self.blocks, self.W)
                self.n_waves = len(self.tiles)
                self.pts = {}

            def score_wave(self, w):
                """One psum tile: its score matmuls, exps, and masks."""
                i0, s = self.i0, self.s
                tile = self.tiles[w]
                st = alloc_score()
                pt = ptp.tile([128, 1024], bf16, name="pt", tag="pt")
                for b, off in tile["placements"]:
                    bwid = b["zhi"] - b["zlo"]
                    nc.tensor.matmul(
                        st[0:b["rows"], off:off + bwid],
                        lhsT=self.kt[self.off:self.off + 64,
                                     b["j0"]:b["j0"] + b["rows"]],
                        rhs=self.qt[self.off:self.off + 64,
                                    i0 + b["zlo"]:i0 + b["zhi"]],
                        start=True, stop=True)
                    self.pts[b["jb"]] = (pt, off, b)
                for exp in tile["exps"]:
                    if exp[0] == "single":
                        _, b, off = exp
                        rows, bwid = b["rows"], b["zhi"] - b["zlo"]
                        if b["bias"]:
                            nc.scalar.activation(
                                pt[0:rows, off:off + bwid],
                                st[0:rows, off:off + bwid], AF.Exp,
                                bias=consts[0:rows,
                                            CONST_BIAS + s:CONST_BIAS + s + 1],
                                scale=1.0)
                        else:
                            nc.scalar.activation(
                                pt[0:rows, off:off + bwid],
                                st[0:rows, off:off + bwid], AF.Exp)
                    elif exp[0] == "run":
                        _, blks, off, wtot = exp
                        nc.scalar.activation(
                            pt[:, off:off + wtot], st[:, off:off + wtot],
                            AF.Exp)
                    else:  # strided
                        _, blks, off0, stride, wmax = exp
                        s0, ng = off0 // stride, len(blks)
                        nc.scalar.activation(
                            pt[:].rearrange("p (g c) -> p g c", c=stride)
                            [:, s0:s0 + ng, 0:wmax],
                            st[:].rearrange("p (g c) -> p g c", c=stride)
                            [:, s0:s0 + ng, 0:wmax],
                            AF.Exp)
                for b, off in tile["placements"]:
                    if b["mask"] is not None:
                        mid, c0, c1 = b["mask"]
                        mo, mw = mask_offs[mid]
                        mask_rr[0] += 1
                        if self.kind == "seq":
                            eng = (nc.gpsimd if mask_rr[0] % 4 == 0
                                   else nc.vector)
                        else:  # alternate loc masks DVE/Pool
                            eng = (nc.gpsimd if mask_rr[0] % 2
                                   else nc.vector)
                        o0 = off + c0 - b["zlo"]
                        eng.tensor_mul(
                            pt[0:b["rows"], o0:o0 + mw],
                            pt[0:b["rows"], o0:o0 + mw],
                            maskcat[0:b["rows"], mo:mo + mw])

            def av_qgroup(self, glo, rows_qg, yt, pos):
                """Accumulate this softmax's AV^T for chunk-relative queries
                [glo, glo+rows_qg) into yt psum cols [pos*65, pos*65+65).

                Output partitions are queries; column 64-of-65 collects the
                softmax denominator via the ones column in vext. blocks[0]
                covers [0, W) so the start=True matmul spans all rows; later
                (partial) blocks always satisfy zlo <= glo (staircase aligns
                with the 128 query grid) and accumulate row subranges."""
                ghi = glo + rows_qg
                blks = [b for b in self.blocks
                        if max(b["zlo"], glo) < min(b["zhi"], ghi)]
                for bi, b in enumerate(blks):
                    assert b["zlo"] <= glo, (self.s, self.ici, glo, b["zlo"])
                    hi = min(b["zhi"], ghi)
                    pt, off, _ = self.pts[b["jb"]]
                    nc.tensor.matmul(
                        yt[0:hi - glo, pos * 65:pos * 65 + 65],
                        lhsT=pt[0:b["rows"],
                                off + glo - b["zlo"]:off + hi - b["zlo"]],
                        rhs=vext[b["jb"]][0:b["rows"],
                                          self.hv * 65:self.hv * 65 + 65],
                        start=(bi == 0), stop=(bi == len(blks) - 1))

        mask_rr = [0]

        # merged output staging: one tile per trio of token chunks
        ost3 = [P.tile([128, 1536], f32, name=f"ost{i}", tag=f"ost{i}")
                for i in range(3)]

        def out_proj(m, ytq, JHt):
            """Out-projection for token chunk m (= query group m).

            ytq: compact transposed tile [128, cc(4), JHt] (c = cc*128+p)."""
            j0, JH = JBS[m]
            yqv = ytq[:].rearrange("p (c t) -> p c t", t=JHt)
            trio, slot = divmod(m, 3)
            po = alloc_small()
            for p in range(4):
                nc.tensor.matmul(
                    po[0:JH, :],
                    lhsT=yqv[:, p, 0:JH],
                    rhs=wp_sb[:, p * 512:(p + 1) * 512],
                    start=(p == 0), stop=(p == 3))
            nc.vector.tensor_copy(ost3[trio][0:JH, slot * 512:slot * 512 + 512],
                                  po[0:JH, :])
            if trio == 2:  # final trio: DMA each block immediately (tail)
                nc.sync.dma_start(
                    out_ap[j0:j0 + JH, :],
                    ost3[trio][0:JH, slot * 512:slot * 512 + 512])
            elif slot == 2:  # trio complete -> one merged DMA
                t0 = trio * 384
                ov = ost3[trio][:].rearrange("p (s c) -> p s c", c=512)
                nc.sync.dma_start(
                    out_ap[t0:t0 + 384, :].rearrange("(s p) c -> p s c", s=3),
                    ov[:, 0:3])

        def norm_qg(ici, glo, rows_qg, ytg, qg_global):
            """Normalize one query group from its ytg staging and fill yTn.

            rz[:, s] = f_s / Z_s per query partition; y_norm = ytg * rz
            broadcast; ml components scaled by w_l are added into mixed
            heads 2/3; yTn gets the [c, token] layout via DMA transpose
            (PE transpose for the 2-token tail)."""
            ytgv = ytg[0:rows_qg].rearrange("p (s e) -> p s e", e=65)
            rz = rzp.tile([128, 16], bf16, name="rz", tag="rz")
            ctx2 = nc.allow_low_precision(reason="bf16 softmax normalization")
            ctx2.__enter__()
            nc.vector.reciprocal(rz[0:rows_qg, 0:NSM], ytgv[:, :, 64])
            nc.vector.tensor_tensor(
                rz[0:rows_qg, 0:NSM], rz[0:rows_qg, 0:NSM],
                fcol_bf[0:rows_qg, 0:NSM], op=MUL)
            yn = ynp.tile([128, 512], bf16, name="yn", tag="yn")
            mlt = mltp.tile([128, 128], bf16, name="mlt", tag="mlt")
            ynv = yn[0:rows_qg].rearrange("p (s e) -> p s e", e=64)
            mlv = mlt[0:rows_qg].rearrange("p (s e) -> p s e", e=64)
            nc.vector.tensor_tensor(
                ynv[:, 0:8], ytgv[:, 0:8, 0:64],
                rz[0:rows_qg, 0:8, None].broadcast_to((rows_qg, 8, 64)),
                op=MUL)
            nc.vector.tensor_tensor(
                mlv[:, 0:2], ytgv[:, 8:10, 0:64],
                rz[0:rows_qg, 8:10, None].broadcast_to((rows_qg, 2, 64)),
                op=MUL)
            nc.vector.tensor_tensor(yn[0:rows_qg, 128:192],
                                    yn[0:rows_qg, 128:192],
                                    mlt[0:rows_qg, 0:64], op=ADD)
            nc.vector.tensor_tensor(yn[0:rows_qg, 192:256],
                                    yn[0:rows_qg, 192:256],
                                    mlt[0:rows_qg, 64:128], op=ADD)
            if DEBUG_TAPS and qg_global == DEBUG_QG:
                dbg_ytg = P.tile([128, 650], bf16, name="dytg", tag="dytg")
                dbg_yn = P.tile([128, 512], bf16, name="dyn", tag="dyn")
                nc.vector.tensor_copy(dbg_ytg[0:rows_qg, :], ytg[0:rows_qg, :])
                nc.vector.tensor_copy(dbg_yn[0:rows_qg, :], yn[0:rows_qg, :])
                for nm, t in (("ytgq", dbg_ytg), ("ynq", dbg_yn)):
                    dst = nc.dram_tensor(f"dbg_{nm}", [128, t.shape[1]],
                                         t[:].dtype, kind="ExternalOutput").ap()
                    nc.sync.dma_start(dst[:, :], t[:])
            if rows_qg >= 16:
                # one transpose DMA -> compact [128, 4, rows] tile
                # (out[p, cc, t] = yn[t, cc*128+p]; out must be contiguous)
                ytq = ytqp.tile([128, 512], bf16, name="ytq", tag="ytq")
                nc.sync.dma_start_transpose(
                    ytq[:].rearrange("p (c t) -> p c t", t=rows_qg),
                    yn[0:rows_qg, :])
                out_proj(qg_global, ytq, rows_qg)
            else:  # 2-token tail: PE transpose through a bf16 psum tile
                tps = wp4.tile([128, 1024], bf16, name="tp", tag="wp4")
                for cc in range(4):
                    nc.tensor.transpose(
                        tps[:, cc * 2:cc * 2 + 2],
                        yn[0:rows_qg, cc * 128:(cc + 1) * 128], eye2)
                ytq = ytqp.tile([128, 512], bf16, name="ytq", tag="ytq")
                nc.vector.tensor_copy(ytq[:, 0:4 * rows_qg], tps[:, 0:8])
                out_proj(qg_global, ytq, rows_qg)
            ctx2.__exit__(None, None, None)

        def process_chunk(ici, fillers, last=False):
            """All 10 softmaxes of one query chunk, in two 5-softmax gens.

            Per softmax: score waves -> (fillers) -> next chunk's waves for
            the same softmax (prebuild: fills ACT gaps early so the final
            chunk is AV/norm-only) -> previous softmax's AV^T. Gen g's Y^T
            psum tiles (one bank per qgroup) evacuate into ytg when the
            gen's last softmax has AV'd (on ACT for the last chunk, where
            ACT is otherwise drained)."""
            qgs = chunk_qgroups(ici)
            base_qg = sum(len(chunk_qgroups(i)) for i in range(ici))
            ytg_t = [ytgp.tile([128, 650], bf16, name="ytg", tag="ytg")
                     for _ in qgs]
            gen_tiles = {}
            pend = None

            def flush(pend_ch):
                ch, gi = pend_ch
                if gi not in gen_tiles:
                    gen_tiles[gi] = [
                        wp4.tile([128, 512], f32, name="yt", tag="wp4")
                        for _ in qgs]
                for qi, (glo, rows_qg) in enumerate(qgs):
                    ch.av_qgroup(glo, rows_qg, gen_tiles[gi][qi],
                                 POS[ch.s] % 5)
                if ch.s == GEN_SMS[gi][-1]:  # gen complete -> evacuate
                    with nc.allow_low_precision(reason="bf16 ytg staging"):
                        for qi, (glo, rows_qg) in enumerate(qgs):
                            dst = ytg_t[qi][0:rows_qg,
                                            gi * 325:gi * 325 + 325]
                            src = gen_tiles[gi][qi][0:rows_qg, 0:325]
                            if last:
                                nc.scalar.activation(dst, src, AF.Copy)
                            else:
                                nc.vector.tensor_copy(dst, src)

            for gi, sms in enumerate(GEN_SMS):
                for s in sms:
                    si = POS[s]
                    ch = Chunk(s, ici)
                    for w in range(ch.n_waves):
                        ch.score_wave(w)
                    for f in fillers.get(si, []):
                        f()
                    if pend is not None:
                        flush(pend)
                    pend = (ch, gi)
            flush(pend)
            # defer norms/out-projs into the next chunk's slots so their
            # psum/pool allocations trail the next chunk's gen tiles
            return [(lambda glo=glo, rows_qg=rows_qg, t=t, q=q:
                     norm_qg(ici, glo, rows_qg, t, q))
                    for (glo, rows_qg), t, q in
                    zip(qgs, ytg_t, range(base_qg, base_qg + len(qgs)))]

        # ---------------- emission schedule ----------------
        # Chunk-major. Projections for chunk ici+1 ride as fillers inside
        # chunk ici; all of chunk 0's own projections are emitted up front /
        # in its first softmax slots (DMA-gated anyway).
        fillers0 = {
            0: [lambda: proj_v(0), lambda: proj_v(1),
                lambda: proj_qk1(1, 0, "q"), lambda: proj_qk1(1, 0, "k")],
            1: [lambda: proj_v(2), lambda: proj_v(3),
                lambda: proj_qk1(2, 0, "q"), lambda: proj_qk1(2, 0, "k")],
            2: [lambda: proj_qk1(3, 0, "q"), lambda: proj_qk1(3, 0, "k"),
                lambda: proj_v(4)],
            3: [lambda: proj_ml1(0, "q"), lambda: proj_ml1(0, "k"),
                lambda: proj_v(5)],
            4: [lambda: proj_qk1(0, 1, "q"), lambda: proj_qk1(0, 1, "k"),
                lambda: proj_v(6)],
            5: [lambda: proj_qk1(1, 1, "q"), lambda: proj_qk1(1, 1, "k"),
                lambda: proj_v(7)],
            6: [lambda: proj_qk1(2, 1, "q"), lambda: proj_qk1(2, 1, "k"),
                lambda: proj_v(8)],
            7: [lambda: proj_qk1(3, 1, "q"), lambda: proj_qk1(3, 1, "k")],
            8: [lambda: proj_ml1(1, "q"), lambda: proj_ml1(1, "k")],
            9: [lambda: proj_qk1(0, 2, "q"), lambda: proj_qk1(0, 2, "k")],
        }
        fillers1 = {
            0: [lambda: proj_qk1(1, 2, "q"), lambda: proj_qk1(1, 2, "k")],
            1: [lambda: proj_qk1(2, 2, "q"), lambda: proj_qk1(2, 2, "k")],
            2: [lambda: proj_qk1(3, 2, "q"), lambda: proj_qk1(3, 2, "k")],
            3: [lambda: proj_ml1(2, "q"), lambda: proj_ml1(2, "k")],
        }
        proj_qk1(0, 0, "q")
        proj_qk1(0, 0, "k")
        d0 = process_chunk(0, fillers0)
        for si, d in zip((2, 3, 4), d0):
            fillers1.setdefault(si, []).append(d)
        d1 = process_chunk(1, fillers1)
        fillers2 = {si: [d] for si, d in zip((2, 3, 4), d1)}
        d2 = process_chunk(2, fillers2, last=True)
        for d in d2:
            d()

        if DEBUG_TAPS:
            taps = dict(qT0=qT[0], kT0=kT[0], qml=qml, vx0=vext[0],
                        yTn4=yTn4, xt8=xt8_sb)
            for nm in DEBUG_TAPS:
                t = taps[nm]
                shp = [t.shape[0], t.shape[1]]
                dt_ = t[:].dtype
                dst = nc.dram_tensor(f"dbg_{nm}", shp, dt_,
                                     kind="ExternalOutput").ap()
                nc.sync.dma_start(dst[:, :], t[:])


# ----------------------------------------------------------------------------
# module build + run
# ----------------------------------------------------------------------------
_CACHE = {}


def _get_module():
    if "nc" in _CACHE:
        return _CACHE["nc"], _CACHE["plans"], _CACHE["mask_offs"], _CACHE["maskcat"]
    import concourse.tile as tile
    from concourse import bacc, mybir

    plans, maskcat, mask_offs = build_block_plan()
    eye = np.zeros((128, 2), np.float32)
    eye[0, 0] = eye[1, 1] = 1.0
    maskcat = (np.concatenate([maskcat, eye], axis=1)
               if maskcat.shape[1] else eye)
    mask_w = maskcat.shape[1]

    nc = bacc.Bacc("TRN2", target_bir_lowering=False, debug=False,
                   enable_asserts=False, num_devices=NCORES)
    f32 = mybir.dt.float32
    bf16 = mybir.dt.bfloat16
    f8 = mybir.dt.float8e4

    def din(name, shape, dt=f32):
        return nc.dram_tensor(name, list(shape), dt, kind="ExternalInput").ap()

    ins = dict(
        xt=din("xt", (EMBED, T), bf16),
        xt8=din("xt8", (128, 4 * T), f8),
        wqk=din("wqk", (128, 4096), f8),
        wv=din("wv", (128, 2048), bf16),
        wml=din("wml", (128, 1024), f8),
        wp=din("wp", (128, 2048), f32),
        consts=din("consts", (128, CONST_W), f32),
        masks=din("masks", (128, mask_w), bf16),
    )
    out_ap = nc.dram_tensor("out_p", [T, EMBED], f32, kind="ExternalOutput").ap()

    with tile.TileContext(nc) as tc:
        emit_kernel(tc, ins, out_ap, plans, mask_offs, mask_w)
    nc.compile()

    _CACHE.update(nc=nc, plans=plans, mask_offs=mask_offs, maskcat=maskcat)
    return nc, plans, mask_offs, maskcat


def build_in_maps(inputs):
    nc, plans, mask_offs, maskcat = _get_module()
    x = inputs["x"].astype(np.float32)
    cond = np.asarray(inputs["cond_mask"]).astype(np.int32)
    B = x.shape[0]
    assert B == NCORES, f"expected B={NCORES}, got {B}"

    ws = prep_weights(inputs)  # weights may differ between calls
    if "masks_bf" not in _CACHE:  # masks are static problem constants
        mc = maskcat if maskcat.shape[1] else np.zeros((128, 2), np.float32)
        _CACHE["masks_bf"] = mc.astype(BF16)
    perm, _ = build_perm()

    in_maps = []
    bias_cache = {}
    for b in range(B):
        cb = int(cond[b])
        if cb not in bias_cache:
            consts = ws["consts"].copy()
            consts[:, CONST_BIAS:CONST_BIAS + NSM] = core_biascols(inputs, cb)
            bias_cache[cb] = consts
        xtb = np.ascontiguousarray(x[b][perm].T)  # [512, T]
        xt8 = np.ascontiguousarray(
            xtb.reshape(4, 128, T).transpose(1, 0, 2).reshape(128, 4 * T)
        ).astype(F8)
        in_maps.append(dict(
            xt=xtb.astype(BF16), xt8=xt8,
            wqk=ws["wqk"], wv=ws["wv"], wml=ws["wml"], wp=ws["wp"],
            consts=bias_cache[cb], masks=_CACHE["masks_bf"],
        ))
    return nc, in_maps


def kernel(**inputs):
    from concourse import bass_utils

    inputs = {k: np.asarray(v) for k, v in inputs.items()}
    nc, in_maps = build_in_maps(inputs)
    res = bass_utils.run_bass_kernel_spmd(nc, in_maps, core_ids=list(range(NCORES)))
    _CACHE["last_results"] = res

    _, inv = build_perm()
    shift = host_const_shift(inputs)
    B = inputs["x"].shape[0]
    out = np.empty((B, T, EMBED), dtype=np.float32)
    for b in range(B):
        out[b] = res.results[b]["out_p"][inv] + shift
    return out



# revision 58
# speedup vs baseline: 1.0181x; 1.0052x over previous
"""Trainium2 Bass kernel for nn_CausalCrossConditionalSelfAttention.

Data-parallel over batch B=8, one element per core. Design:
  - Exact T=1026 (no padding): query chunks (384,386,256), key blocks
    8x128 + one 2-row tiny block; scores/exp/AV operate on per-block
    column ranges [zlo,zhi) so the causal triangle / local band is not
    padded to full chunk width.
  - bf16 for x/weights/qT/kT/pt/vext/masks (halves DMA, 2x DVE mask-muls,
    any-N matmuls); fp32 psums, Z path, and final out-projection.
  - ~15 DMAs total (each DMACopy costs ~630ns on the shared HWDGE).
  - Softmax denominators ride as a ones-column in the AV matmul; Z rows are
    scaled by 1/mix-factor into a [65,W] staging tile (partitions 0/64),
    broadcast to 128 partitions by one select-matmul per head-pair, and
    applied via reciprocal + in-place multiply (mix weights folded in).
  - Score blocks bin-packed into [128,1024] psum tiles (256-wide slots for
    the 256 chunk, contiguous runs for narrow local-band blocks, stride-512
    pairs for wide blocks) to minimize exp instruction count.
  - One global software-pipelined emission pass: chunk n's score waves
    interleave with chunk n-2's AV matmuls; projections and out-projections
    fill PE slack; per-head-pair normalization fires as soon as both
    members finish.
"""

import sys

if "/opt/trn_rl_repo" not in sys.path:
    sys.path.insert(0, "/opt/trn_rl_repo")

import numpy as np

try:
    import ml_dtypes
    BF16 = np.dtype(ml_dtypes.bfloat16)
    F8 = np.dtype(ml_dtypes.float8_e4m3)
except ImportError:  # pragma: no cover
    BF16 = None
    F8 = None

# fp8 weight scale: w*scale values (~0.0025) sit in e4m3's subnormal range,
# so store w*SW and multiply psum by 1/SW in the evacuation op.
SW = 256.0
DEBUG_TAPS = ()
DEBUG_QG = 1

# ----------------------------------------------------------------------------
# problem constants
# ----------------------------------------------------------------------------
BLOCK = 512
RECEP = 4
N_HEAD = 8
EMBED = 512
HS = 64
T = 2 * BLOCK + 2          # 1026
NSM = 10
NCORES = 8

# query chunks (offset, width): 128-aligned starts so the causal staircase's
# block zlo values never land inside a 128-query group (AV out base always 0)
ICS = [(0, 384), (384, 384), (768, 258)]
# key blocks (offset, height)
JBS = [(j * 128, 128) for j in range(8)] + [(1024, 2)]

# softmax id -> (mask kind, q/k source, v head)
SM_INFO = [
    (0, "loc", "main", 0), (1, "loc", "main", 1),
    (2, "seq", "main", 2), (3, "seq", "main", 3),
    (4, "seq", "main", 4), (5, "seq", "main", 5),
    (6, "seq", "main", 6), (7, "seq", "main", 7),
    (8, "loc", "ml", 2), (9, "loc", "ml", 3),
]
# softmax emission generations per chunk: Y^T psum gen tiles hold 5 softmaxes
# (5*65=325 cols, one bank); ytg staging column position of softmax s
GEN_SMS = [[0, 1, 2, 3, 4], [5, 6, 7, 8, 9]]
POS = {s: gi * 5 + i for gi, g in enumerate(GEN_SMS) for i, s in enumerate(g)}


def chunk_qgroups(ici):
    i0, W = ICS[ici]
    return [(g * 128, min(128, W - g * 128)) for g in range((W + 127) // 128)]


# ----------------------------------------------------------------------------
# host-side plan construction
# ----------------------------------------------------------------------------
def build_perm():
    perm = np.zeros(T, dtype=np.int64)
    perm[0], perm[1] = 0, 1
    b = np.arange(BLOCK)
    perm[2 + 2 * b] = 2 + b
    perm[3 + 2 * b] = 2 + BLOCK + b
    inv = np.argsort(perm)
    return perm, inv


def build_masks_orig():
    to = np.concatenate([np.zeros(2), np.arange(BLOCK) * 2 + 1, np.arange(BLOCK) * 2 + 2])
    seq = to[None, :] <= to[:, None]
    qo = np.concatenate([np.arange(BLOCK) * 2 + 1 - 2 * RECEP + 1] * 2)
    ko = np.concatenate([np.arange(BLOCK) * 2 + 1] * 2)
    de = ko[None, :] < qo[:, None]
    loc = seq.copy()
    loc[2:, 2:] = loc[2:, 2:] & (~de)
    return seq, loc


def build_block_plan():
    """Per (kind, ic): list of block dicts with exact column ranges.

    block = dict(jb, j0, rows, zlo, zhi, bias, mask=(mid,c0,c1) or None)
    Ordered so the first block covers [0, W) (widest) for PSUM start=True.
    """
    perm, _ = build_perm()
    seq, loc = build_masks_orig()
    Ms = seq[perm][:, perm]
    Ml = loc[perm][:, perm]

    mask_tiles = []
    tile_index = {}

    def tile_id(tile):
        key = tile.tobytes() + bytes(str(tile.shape), "ascii")
        if key not in tile_index:
            tile_index[key] = len(mask_tiles)
            mask_tiles.append(tile)
        return tile_index[key]

    plans = {}
    for kind, M in (("seq", Ms), ("loc", Ml)):
        plan = []
        for i0, W in ICS:
            blocks = []
            for jb, (j0, JH) in enumerate(JBS):
                sub = M[i0:i0 + W, j0:j0 + JH].T  # [JH, W] keys x queries
                if not sub.any():
                    continue
                nz_rows = np.flatnonzero(sub.any(axis=1))
                rows = int(nz_rows.max()) + 1
                colmask = sub[:rows].any(axis=0)
                nz_cols = np.flatnonzero(colmask)
                zlo, zhi = int(nz_cols.min()), int(nz_cols.max()) + 1
                core = sub[:rows, zlo:zhi]
                if core.all():
                    mask = None
                else:
                    pc = np.flatnonzero(~core.all(axis=0))
                    c0, c1 = zlo + int(pc.min()), zlo + int(pc.max()) + 1
                    mid = tile_id(
                        sub[:rows, c0:c1].astype(np.float32).copy())
                    mask = (mid, c0, c1)
                blocks.append(dict(jb=jb, j0=j0, rows=rows, zlo=zlo, zhi=zhi,
                                   bias=(j0 == 0), mask=mask))
            # widest-coverage block first (needed for PSUM start=True)
            blocks.sort(key=lambda b: (b["zlo"], -b["zhi"]))
            assert blocks[0]["zlo"] == 0 and blocks[0]["zhi"] == W, (kind, i0)
            plan.append(blocks)
        plans[kind] = plan

    offs, cat = [], []
    o = 0
    for t in mask_tiles:
        offs.append((o, t.shape[1]))
        cat.append(np.pad(t, ((0, 128 - t.shape[0]), (0, 0))))
        o += t.shape[1]
    maskcat = (np.concatenate(cat, axis=1) if cat
               else np.zeros((128, 0), np.float32))
    return plans, maskcat, offs


def build_exp_tiles(blocks, W):
    """Pack a chunk's blocks into [128,1024] score-psum tiles.

    Returns a list of tiles; each tile is a dict:
      placements: [(block, off)]          off in [0,1024), bank-contained
      exps: [("single", block, off)]      bias / tiny blocks
            [("run", [blocks], off, w)]   contiguous narrow blocks, one bank
            [("strided", [blocks], off0, stride, wmax)]
    """
    def bw(b):
        return b["zhi"] - b["zlo"]

    specials = [b for b in blocks if b["bias"] or b["rows"] < 128]
    plain = sorted((b for b in blocks if not (b["bias"] or b["rows"] < 128)),
                   key=lambda b: b["jb"])
    tiles = []

    def new_tile():
        tiles.append(dict(placements=[], exps=[], used=0))
        return tiles[-1]

    if W <= 256:
        # uniform 256-wide slots, 4 per tile; strided exps over plain runs
        slots = specials + plain  # bias first, then jb order
        t = None
        for i, b in enumerate(slots):
            si = i % 4
            if si == 0:
                t = new_tile()
            t["placements"].append((b, si * 256))
        # exps: walk slots; specials single, plain grouped per tile
        for ti, t in enumerate(tiles):
            runb, ro, wmax = [], 0, 0
            for b, off in t["placements"]:
                if b["bias"] or b["rows"] < 128:
                    t["exps"].append(("single", b, off))
                else:
                    if not runb:
                        ro = off
                    runb.append(b)
                    wmax = max(wmax, bw(b))
            if runb:
                t["exps"].append(("strided", runb, ro, 256, wmax))
        return tiles

    wide = [b for b in plain if bw(b) > 256]
    narrow = [b for b in plain if bw(b) <= 256]
    # wide: stride-512 pairs occupying a full tile; narrow leftovers are
    # appended into pair spare bank space, extending the exp width (the
    # shorter bank's tail exps stale psum, which is never read downstream)
    pairs = []
    i = 0
    while i < len(wide):
        t = new_tile()
        pair = wide[i:i + 2]
        ext = []
        for g, b in enumerate(pair):
            t["placements"].append((b, g * 512))
            ext.append(bw(b))
        t["used"] = 2
        pairs.append((t, pair, ext))
        i += 2
    rem = []
    for b in narrow:
        placed = False
        for t, pair, ext in pairs:
            for k in sorted(range(len(pair)), key=lambda k: ext[k]):
                if ext[k] + bw(b) <= 512:
                    t["placements"].append((b, k * 512 + ext[k]))
                    ext[k] += bw(b)
                    placed = True
                    break
            if placed:
                break
        if not placed:
            rem.append(b)
    narrow = rem
    for t, pair, ext in pairs:
        if len(pair) == 2:
            t["exps"].append(("strided", pair, 0, 512, max(ext)))
        else:
            t["exps"].append(("run", pair, 0, ext[0]))

    free_banks = []
    def alloc_bank():
        if not free_banks:
            t = new_tile()
            t["used"] = 2
            free_banks.extend([(t, 0), (t, 512)])
        return free_banks.pop(0)

    if narrow:
        run, runw = [], 0
        bank = alloc_bank()
        for b in narrow:
            if runw + bw(b) > 512:
                t, boff = bank
                t["exps"].append(("run", run, boff, runw))
                bank = alloc_bank()
                run, runw = [], 0
            t, boff = bank
            t["placements"].append((b, boff + runw))
            run.append(b)
            runw += bw(b)
        t, boff = bank
        t["exps"].append(("run", run, boff, runw))
    for b in specials:
        bank = alloc_bank()
        t, boff = bank
        t["placements"].append((b, boff))
        t["exps"].append(("single", b, boff))
    return tiles


# ----------------------------------------------------------------------------
# host-side input prep
# ----------------------------------------------------------------------------
# consts tile layout (fp32, [128, CW]):
#   [0:4)   bq per m-chunk      [4:8) bk
#   [8]     bqml                [9]   bkml
#   [10:20) biascols (exp bias per softmax)
#   [20:30) f_s mix factor per softmax (all partitions; 1 except sm 2,3,8,9)
CONST_BQ, CONST_BK, CONST_BQML, CONST_BKML = 0, 4, 8, 9
CONST_BIAS = 10
CONST_F = 20
CONST_W = 30


def prep_weights(w):
    """Shared (per-batch-invariant) device buffers."""
    f = np.float32
    scale = f(1.0 / np.sqrt(HS))

    wqT = w["w_query"].astype(f).T * scale     # [cin, cout]
    wkT = w["w_key"].astype(f).T
    wvT = w["w_value"].astype(f).T
    wpT = w["w_proj"].astype(f).T
    wqmlT = w["w_query_ml"].astype(f).T * scale  # [512, 128]
    wkmlT = w["w_key_ml"].astype(f).T

    # wqk8: fp8 DoubleRow layout [128, kc(4), 1024] -> [128, 4096]
    # [p, kc, c] = (wq|wk).T[kc*128+p, c] * SW
    wqk = np.ascontiguousarray(
        (np.concatenate([wqT, wkT], axis=1) * SW)
        .reshape(4, 128, 1024).transpose(1, 0, 2).reshape(128, 4096)
    ).astype(F8)
    # wv single tile [128, 4*512]: [p, kc*512+c] = wvT[kc*128+p, c]
    wv = np.ascontiguousarray(
        wvT.reshape(4, 128, 512).transpose(1, 0, 2).reshape(128, 2048)
    ).astype(BF16)
    # wml8 fp8 DR tile [128, 4*256]: per kc [qml 128 | kml 128]
    wml = np.ascontiguousarray(
        (np.concatenate([wqmlT.reshape(4, 128, 128),
                         wkmlT.reshape(4, 128, 128)], axis=2) * SW)
        .transpose(1, 0, 2).reshape(128, 1024)
    ).astype(F8)
    # wp bf16 single tile [128, 4*512] (pairs with bf16 yTn in out-proj)
    wp = np.ascontiguousarray(
        wpT.reshape(4, 128, 512).transpose(1, 0, 2).reshape(128, 2048)
    ).astype(BF16)

    # consts (biascols filled per core)
    consts = np.zeros((128, CONST_W), dtype=f)
    consts[:, CONST_BQ:CONST_BQ + 4] = (w["b_query"].astype(f) * scale
                                        ).reshape(4, 128).T
    consts[:, CONST_BK:CONST_BK + 4] = w["b_key"].astype(f).reshape(4, 128).T
    consts[:, CONST_BQML] = (w["b_query_ml"].astype(f) * scale)
    consts[:, CONST_BKML] = w["b_key_ml"].astype(f)

    wg = w["w_mix"].astype(f)[:, 0, 0, 0]
    wl = w["w_mix"].astype(f)[:, 1, 0, 0]
    fs = np.ones(NSM, dtype=f)
    fs[2], fs[3] = wg[0], wg[1]
    fs[8], fs[9] = wl[0], wl[1]
    consts[:, CONST_F:CONST_F + NSM] = fs[None, :]
    return dict(wqk=wqk, wv=wv, wml=wml, wp=wp, consts=consts)


def core_biascols(w, cond_b):
    f = np.float32
    bias = np.zeros((128, NSM), dtype=f)
    if cond_b > 0:
        clip8 = np.maximum(w["att_bias_clip"].astype(f)[0, :, 0], 0.0) * 10.0
        clip2 = np.maximum(w["att_bias_clip_ml"].astype(f)[0, :, 0], 0.0) * 10.0
        bias[1, :N_HEAD] = clip8
        bias[1, N_HEAD:] = clip2
    return bias


def host_const_shift(w):
    bv = w["b_value"].astype(np.float64)
    wg = w["w_mix"].astype(np.float64)[:, 0, 0, 0]
    wl = w["w_mix"].astype(np.float64)[:, 1, 0, 0]
    scale_h = np.ones(N_HEAD)
    scale_h[2] = wg[0] + wl[0]
    scale_h[3] = wg[1] + wl[1]
    yshift = (bv.reshape(N_HEAD, HS) * scale_h[:, None]).reshape(-1)
    return (yshift @ w["w_proj"].astype(np.float64).T
            + w["b_proj"].astype(np.float64)).astype(np.float32)


# ----------------------------------------------------------------------------
# bass kernel emission
# ----------------------------------------------------------------------------
def emit_kernel(tc, ins, out_ap, plans, mask_offs, mask_w):
    from contextlib import ExitStack
    from concourse import mybir

    nc = tc.nc
    f32 = mybir.dt.float32
    f32r = mybir.dt.float32r
    bf16 = mybir.dt.bfloat16
    AF = mybir.ActivationFunctionType

    def r(ap):
        return ap.bitcast(f32r)

    with ExitStack() as ctx:
        P = ctx.enter_context(tc.tile_pool(name="persist", bufs=1))

        # ---------------- persistent SBUF tiles ----------------
        f8 = mybir.dt.float8e4
        xT = [P.tile([128, T], bf16, name=f"x{k}", tag=f"x{k}") for k in range(4)]
        xt8_sb = P.tile([128, 4 * T], f8, name="xt8", tag="xt8")
        wqk8_sb = P.tile([128, 4096], f8, name="wqk8", tag="wqk8")
        wv_sb = P.tile([128, 2048], bf16, name="wv", tag="wv")
        wml_sb = P.tile([128, 1024], f8, name="wml", tag="wml")
        wp_sb = P.tile([128, 2048], bf16, name="wp", tag="wp")
        consts = P.tile([128, CONST_W], f32, name="consts", tag="consts")
        maskcat = P.tile([128, mask_w], bf16, name="maskcat", tag="maskcat")
        # DoubleRow-ready views [p, kc, cols]
        x8v = xt8_sb[:].rearrange("p (k c) -> p k c", c=T)
        w8v = wqk8_sb[:].rearrange("p (k c) -> p k c", c=1024)
        wml8v = wml_sb[:].rearrange("p (k c) -> p k c", c=256)
        DR = mybir.MatmulPerfMode.DoubleRow

        qT = [P.tile([128, T], bf16, name=f"qT{m}", tag=f"qT{m}") for m in range(4)]
        kT = [P.tile([128, T], bf16, name=f"kT{m}", tag=f"kT{m}") for m in range(4)]
        qml = P.tile([128, T], bf16, name="qml", tag="qml")
        kml = P.tile([128, T], bf16, name="kml", tag="kml")
        vext = [P.tile([128, N_HEAD * 65], bf16, name=f"vx{t}", tag=f"vx{t}")
                for t in range(9)]


        # ---------------- DMA loads ----------------
        # All on the SP queue (HWDGE/DMA-device serialize transfers anyway;
        # keeping ACT's sequencer free for exps). Order = need order.
        nc.sync.dma_start(r(consts[:]), r(ins["consts"][:, :]))
        nc.sync.dma_start(wqk8_sb[:], ins["wqk"][:, :])
        nc.sync.dma_start(xt8_sb[:], ins["xt8"][:, :])
        nc.sync.dma_start(maskcat[:], ins["masks"][:, :])
        nc.sync.dma_start(xT[0][:], ins["xt"][0:128, :])
        nc.sync.dma_start(xT[1][:], ins["xt"][128:256, :])
        nc.sync.dma_start(wv_sb[:], ins["wv"][:, :])
        nc.sync.dma_start(xT[2][:], ins["xt"][256:384, :])
        nc.sync.dma_start(xT[3][:], ins["xt"][384:512, :])
        nc.sync.dma_start(wml_sb[:], ins["wml"][:, :])
        nc.sync.dma_start(wp_sb[:], ins["wp"][:, :])

        # ones columns for the Z row of every AV matmul
        for tt in range(9):
            vx = vext[tt][:].rearrange("p (h e) -> p h e", e=65)
            nc.gpsimd.memset(vx[:, :, 64:65], 1.0)
        # 2x2 identity (tail-transpose operand) rides in the masks buffer
        eye2 = maskcat[0:2, mask_w - 2:mask_w]
        # bf16 copy of the mix factors (pairs with bf16 rz in the norm)
        fcol_bf = P.tile([128, 16], bf16, name="fcol", tag="fcol")
        nc.vector.tensor_copy(fcol_bf[:, 0:NSM],
                              consts[:, CONST_F:CONST_F + NSM])

        # tile pools (SBUF work tiles)
        ptp = ctx.enter_context(tc.tile_pool(name="ptp", bufs=8))
        ytgp = ctx.enter_context(tc.tile_pool(name="ytgp", bufs=4))   # [128,650] f32
        ynp = ctx.enter_context(tc.tile_pool(name="ynp", bufs=4))     # [128,512] bf16
        mltp = ctx.enter_context(tc.tile_pool(name="mltp", bufs=3))
        rzp = ctx.enter_context(tc.tile_pool(name="rzp", bufs=3))
        ytqp = ctx.enter_context(tc.tile_pool(name="ytqp", bufs=3))

        # psum pools: sp (2 x [128,1024] score tiles = 4 banks) + genp
        # (3 x 1-bank long-lived Y^T gen tiles) + smallp (1 bank rotating
        # through transient projection/out-proj/tail tiles) = 8 banks.
        # Long-lived and transient tiles MUST NOT share a pool: rotation
        # could hand a transient a buffer owned by a live gen tile, putting
        # a PE instruction ahead of the AVs that free it (deadlock).
        sp = ctx.enter_context(tc.tile_pool(name="sp", bufs=2, space="PSUM"))
        wp4 = ctx.enter_context(tc.tile_pool(name="wp4", bufs=4, space="PSUM"))
        genp = wp4
        smallp = wp4

        def alloc_score():
            return sp.tile([128, 1024], f32, name="sp", tag="sp")

        def alloc_small():
            return smallp.tile([128, 512], f32, name="smallp", tag="wp4")

        # ---------------- emission helpers ----------------
        MUL, ADD = mybir.AluOpType.mult, mybir.AluOpType.add

        def evac(dst, ps_ap, bcol):
            """psum -> sbuf bf16 with 1/SW rescale + bias add."""
            nc.vector.tensor_scalar(dst, ps_ap, 1.0 / SW,
                                    consts[:, bcol:bcol + 1],
                                    op0=MUL, op1=ADD)

        def proj_qk1(m, ici, which):
            """q or k projection for head-pair m, query chunk ici (fp8 DR).

            Single-psum so the shared wp4 pool holds at most one projection
            tile at a time alongside the three Y^T gen tiles."""
            i0, W = ICS[ici]
            coff = 0 if which == "q" else 512
            ps = alloc_small()
            for j in range(2):
                nc.tensor.matmul(
                    ps[:, 0:W],
                    lhsT=w8v[:, 2 * j:2 * j + 2,
                             coff + m * 128:coff + (m + 1) * 128],
                    rhs=x8v[:, 2 * j:2 * j + 2, i0:i0 + W],
                    start=(j == 0), stop=(j == 1), perf_mode=DR)
            dst_t = qT if which == "q" else kT
            bcol = (CONST_BQ if which == "q" else CONST_BK) + m
            evac(dst_t[m][:, i0:i0 + W], ps[:, 0:W], bcol)

        def proj_ml1(ici, which):
            i0, W = ICS[ici]
            coff, bcol = ((0, CONST_BQML) if which == "q"
                          else (128, CONST_BKML))
            ps = alloc_small()
            for j in range(2):
                nc.tensor.matmul(
                    ps[:, 0:W],
                    lhsT=wml8v[:, 2 * j:2 * j + 2, coff:coff + 128],
                    rhs=x8v[:, 2 * j:2 * j + 2, i0:i0 + W],
                    start=(j == 0), stop=(j == 1), perf_mode=DR)
            dst = (qml if which == "q" else kml)[:, i0:i0 + W]
            evac(dst, ps[:, 0:W], bcol)

        def proj_v(tt):
            j0, JH = JBS[tt]
            ps = alloc_small()
            for kc in range(4):
                nc.tensor.matmul(
                    ps[0:JH, :],
                    lhsT=xT[kc][:, j0:j0 + JH],
                    rhs=wv_sb[:, kc * 512:(kc + 1) * 512],
                    start=(kc == 0), stop=(kc == 3))
            vx = vext[tt][0:JH].rearrange("p (h e) -> p h e", e=65)
            nc.scalar.activation(
                vx[:, :, 0:64], ps[0:JH, :].rearrange("p (h d) -> p h d", d=64),
                AF.Copy)

        class Chunk:
            """One (softmax, query-chunk): score waves -> per-qgroup AV^T."""

            def __init__(self, s, ici):
                self.s, self.ici = s, ici
                _, self.kind, src_, self.hv = SM_INFO[s]
                self.i0, self.W = ICS[ici]
                if src_ == "main":
                    self.qt, self.kt = qT[s // 2], kT[s // 2]
                    self.off = (s % 2) * 64
                else:
                    self.qt, self.kt, self.off = qml, kml, (s - N_HEAD) * 64
                self.blocks = plans[self.kind][ici]
                self.tiles = build_exp_tiles(self.blocks, self.W)
                self.n_waves = len(self.tiles)
                self.pts = {}

            def score_wave(self, w):
                """One psum tile: its score matmuls, exps, and masks."""
                i0, s = self.i0, self.s
                tile = self.tiles[w]
                st = alloc_score()
                pt = ptp.tile([128, 1024], bf16, name="pt", tag="pt")
                for b, off in tile["placements"]:
                    bwid = b["zhi"] - b["zlo"]
                    nc.tensor.matmul(
                        st[0:b["rows"], off:off + bwid],
                        lhsT=self.kt[self.off:self.off + 64,
                                     b["j0"]:b["j0"] + b["rows"]],
                        rhs=self.qt[self.off:self.off + 64,
                                    i0 + b["zlo"]:i0 + b["zhi"]],
                        start=True, stop=True)
                    self.pts[b["jb"]] = (pt, off, b)
                for exp in tile["exps"]:
                    if exp[0] == "single":
                        _, b, off = exp
                        rows, bwid = b["rows"], b["zhi"] - b["zlo"]
                        if b["bias"]:
                            nc.scalar.activation(
                                pt[0:rows, off:off + bwid],
                                st[0:rows, off:off + bwid], AF.Exp,
                                bias=consts[0:rows,
                                            CONST_BIAS + s:CONST_BIAS + s + 1],
                                scale=1.0)
                        else:
                            nc.scalar.activation(
                                pt[0:rows, off:off + bwid],
                                st[0:rows, off:off + bwid], AF.Exp)
                    elif exp[0] == "run":
                        _, blks, off, wtot = exp
                        nc.scalar.activation(
                            pt[:, off:off + wtot], st[:, off:off + wtot],
                            AF.Exp)
                    else:  # strided
                        _, blks, off0, stride, wmax = exp
                        s0, ng = off0 // stride, len(blks)
                        nc.scalar.activation(
                            pt[:].rearrange("p (g c) -> p g c", c=stride)
                            [:, s0:s0 + ng, 0:wmax],
                            st[:].rearrange("p (g c) -> p g c", c=stride)
                            [:, s0:s0 + ng, 0:wmax],
                            AF.Exp)
                for b, off in tile["placements"]:
                    if b["mask"] is not None:
                        mid, c0, c1 = b["mask"]
                        mo, mw = mask_offs[mid]
                        mask_rr[0] += 1
                        if self.kind == "seq":
                            eng = (nc.gpsimd if mask_rr[0] % 4 == 0
                                   else nc.vector)
                        else:  # alternate loc masks DVE/Pool
                            eng = (nc.gpsimd if mask_rr[0] % 2
                                   else nc.vector)
                        o0 = off + c0 - b["zlo"]
                        eng.tensor_mul(
                            pt[0:b["rows"], o0:o0 + mw],
                            pt[0:b["rows"], o0:o0 + mw],
                            maskcat[0:b["rows"], mo:mo + mw])

            def av_qgroup(self, glo, rows_qg, yt, pos):
                """Accumulate this softmax's AV^T for chunk-relative queries
                [glo, glo+rows_qg) into yt psum cols [pos*65, pos*65+65).

                Output partitions are queries; column 64-of-65 collects the
                softmax denominator via the ones column in vext. blocks[0]
                covers [0, W) so the start=True matmul spans all rows; later
                (partial) blocks always satisfy zlo <= glo (staircase aligns
                with the 128 query grid) and accumulate row subranges."""
                ghi = glo + rows_qg
                blks = [b for b in self.blocks
                        if max(b["zlo"], glo) < min(b["zhi"], ghi)]
                for bi, b in enumerate(blks):
                    assert b["zlo"] <= glo, (self.s, self.ici, glo, b["zlo"])
                    hi = min(b["zhi"], ghi)
                    pt, off, _ = self.pts[b["jb"]]
                    nc.tensor.matmul(
                        yt[0:hi - glo, pos * 65:pos * 65 + 65],
                        lhsT=pt[0:b["rows"],
                                off + glo - b["zlo"]:off + hi - b["zlo"]],
                        rhs=vext[b["jb"]][0:b["rows"],
                                          self.hv * 65:self.hv * 65 + 65],
                        start=(bi == 0), stop=(bi == len(blks) - 1))

        mask_rr = [0]

        # merged output staging: one tile per trio of token chunks
        ost3 = [P.tile([128, 1536], f32, name=f"ost{i}", tag=f"ost{i}")
                for i in range(3)]

        def out_proj(m, ytq, JHt):
            """Out-projection for token chunk m (= query group m).

            ytq: compact transposed tile [128, cc(4), JHt] (c = cc*128+p)."""
            j0, JH = JBS[m]
            yqv = ytq[:].rearrange("p (c t) -> p c t", t=JHt)
            trio, slot = divmod(m, 3)
            po = alloc_small()
            for p in range(4):
                nc.tensor.matmul(
                    po[0:JH, :],
                    lhsT=yqv[:, p, 0:JH],
                    rhs=wp_sb[:, p * 512:(p + 1) * 512],
                    start=(p == 0), stop=(p == 3))
            dst = ost3[trio][0:JH, slot * 512:slot * 512 + 512]
            if trio == 2:  # tail: ACT is drained, DVE is not
                nc.scalar.activation(dst, po[0:JH, :], AF.Copy)
            else:
                nc.vector.tensor_copy(dst, po[0:JH, :])
            if trio == 2:  # final trio: DMA each block immediately (tail)
                nc.sync.dma_start(
                    out_ap[j0:j0 + JH, :],
                    ost3[trio][0:JH, slot * 512:slot * 512 + 512])
            elif slot == 2:  # trio complete -> one merged DMA
                t0 = trio * 384
                ov = ost3[trio][:].rearrange("p (s c) -> p s c", c=512)
                nc.sync.dma_start(
                    out_ap[t0:t0 + 384, :].rearrange("(s p) c -> p s c", s=3),
                    ov[:, 0:3])

        def norm_qg(ici, glo, rows_qg, ytg, qg_global):
            """Normalize one query group from its ytg staging and fill yTn.

            rz[:, s] = f_s / Z_s per query partition; y_norm = ytg * rz
            broadcast; ml components scaled by w_l are added into mixed
            heads 2/3; yTn gets the [c, token] layout via DMA transpose
            (PE transpose for the 2-token tail)."""
            ytgv = ytg[0:rows_qg].rearrange("p (s e) -> p s e", e=65)
            rz = rzp.tile([128, 16], bf16, name="rz", tag="rz")
            ctx2 = nc.allow_low_precision(reason="bf16 softmax normalization")
            ctx2.__enter__()
            nc.vector.reciprocal(rz[0:rows_qg, 0:NSM], ytgv[:, :, 64])
            nc.vector.tensor_tensor(
                rz[0:rows_qg, 0:NSM], rz[0:rows_qg, 0:NSM],
                fcol_bf[0:rows_qg, 0:NSM], op=MUL)
            yn = ynp.tile([128, 512], bf16, name="yn", tag="yn")
            mlt = mltp.tile([128, 128], bf16, name="mlt", tag="mlt")
            ynv = yn[0:rows_qg].rearrange("p (s e) -> p s e", e=64)
            mlv = mlt[0:rows_qg].rearrange("p (s e) -> p s e", e=64)
            nc.vector.tensor_tensor(
                ynv[:, 0:8], ytgv[:, 0:8, 0:64],
                rz[0:rows_qg, 0:8, None].broadcast_to((rows_qg, 8, 64)),
                op=MUL)
            nc.vector.tensor_tensor(
                mlv[:, 0:2], ytgv[:, 8:10, 0:64],
                rz[0:rows_qg, 8:10, None].broadcast_to((rows_qg, 2, 64)),
                op=MUL)
            nc.vector.tensor_tensor(yn[0:rows_qg, 128:192],
                                    yn[0:rows_qg, 128:192],
                                    mlt[0:rows_qg, 0:64], op=ADD)
            nc.vector.tensor_tensor(yn[0:rows_qg, 192:256],
                                    yn[0:rows_qg, 192:256],
                                    mlt[0:rows_qg, 64:128], op=ADD)
            if DEBUG_TAPS and qg_global == DEBUG_QG:
                dbg_ytg = P.tile([128, 650], bf16, name="dytg", tag="dytg")
                dbg_yn = P.tile([128, 512], bf16, name="dyn", tag="dyn")
                nc.vector.tensor_copy(dbg_ytg[0:rows_qg, :], ytg[0:rows_qg, :])
                nc.vector.tensor_copy(dbg_yn[0:rows_qg, :], yn[0:rows_qg, :])
                for nm, t in (("ytgq", dbg_ytg), ("ynq", dbg_yn)):
                    dst = nc.dram_tensor(f"dbg_{nm}", [128, t.shape[1]],
                                         t[:].dtype, kind="ExternalOutput").ap()
                    nc.sync.dma_start(dst[:, :], t[:])
            if rows_qg >= 16:
                # one transpose DMA -> compact [128, 4, rows] tile
                # (out[p, cc, t] = yn[t, cc*128+p]; out must be contiguous)
                ytq = ytqp.tile([128, 512], bf16, name="ytq", tag="ytq")
                nc.sync.dma_start_transpose(
                    ytq[:].rearrange("p (c t) -> p c t", t=rows_qg),
                    yn[0:rows_qg, :])
                out_proj(qg_global, ytq, rows_qg)
            else:  # 2-token tail: PE transpose through a bf16 psum tile
                tps = wp4.tile([128, 1024], bf16, name="tp", tag="wp4")
                for cc in range(4):
                    nc.tensor.transpose(
                        tps[:, cc * 2:cc * 2 + 2],
                        yn[0:rows_qg, cc * 128:(cc + 1) * 128], eye2)
                ytq = ytqp.tile([128, 512], bf16, name="ytq", tag="ytq")
                nc.vector.tensor_copy(ytq[:, 0:4 * rows_qg], tps[:, 0:8])
                out_proj(qg_global, ytq, rows_qg)
            ctx2.__exit__(None, None, None)

        def process_chunk(ici, fillers, last=False):
            """All 10 softmaxes of one query chunk, in two 5-softmax gens.

            Per softmax: score waves -> (fillers) -> next chunk's waves for
            the same softmax (prebuild: fills ACT gaps early so the final
            chunk is AV/norm-only) -> previous softmax's AV^T. Gen g's Y^T
            psum tiles (one bank per qgroup) evacuate into ytg when the
            gen's last softmax has AV'd (on ACT for the last chunk, where
            ACT is otherwise drained)."""
            qgs = chunk_qgroups(ici)
            base_qg = sum(len(chunk_qgroups(i)) for i in range(ici))
            ytg_t = [ytgp.tile([128, 650], bf16, name="ytg", tag="ytg")
                     for _ in qgs]
            gen_tiles = {}
            pend = None

            def flush(pend_ch):
                ch, gi = pend_ch
                if gi not in gen_tiles:
                    gen_tiles[gi] = [
                        wp4.tile([128, 512], f32, name="yt", tag="wp4")
                        for _ in qgs]
                for qi, (glo, rows_qg) in enumerate(qgs):
                    ch.av_qgroup(glo, rows_qg, gen_tiles[gi][qi],
                                 POS[ch.s] % 5)
                if ch.s == GEN_SMS[gi][-1]:  # gen complete -> evacuate
                    with nc.allow_low_precision(reason="bf16 ytg staging"):
                        for qi, (glo, rows_qg) in enumerate(qgs):
                            dst = ytg_t[qi][0:rows_qg,
                                            gi * 325:gi * 325 + 325]
                            src = gen_tiles[gi][qi][0:rows_qg, 0:325]
                            if last:
                                nc.scalar.activation(dst, src, AF.Copy)
                            else:
                                nc.vector.tensor_copy(dst, src)

            for gi, sms in enumerate(GEN_SMS):
                for s in sms:
                    si = POS[s]
                    ch = Chunk(s, ici)
                    for w in range(ch.n_waves):
                        ch.score_wave(w)
                    for f in fillers.get(si, []):
                        f()
                    if pend is not None:
                        flush(pend)
                    pend = (ch, gi)
            flush(pend)
            # defer norms/out-projs into the next chunk's slots so their
            # psum/pool allocations trail the next chunk's gen tiles
            return [(lambda glo=glo, rows_qg=rows_qg, t=t, q=q:
                     norm_qg(ici, glo, rows_qg, t, q))
                    for (glo, rows_qg), t, q in
                    zip(qgs, ytg_t, range(base_qg, base_qg + len(qgs)))]

        # ---------------- emission schedule ----------------
        # Chunk-major. Projections for chunk ici+1 ride as fillers inside
        # chunk ici; all of chunk 0's own projections are emitted up front /
        # in its first softmax slots (DMA-gated anyway).
        fillers0 = {
            0: [lambda: proj_v(0), lambda: proj_v(1),
                lambda: proj_qk1(1, 0, "q"), lambda: proj_qk1(1, 0, "k")],
            1: [lambda: proj_v(2), lambda: proj_v(3),
                lambda: proj_qk1(2, 0, "q"), lambda: proj_qk1(2, 0, "k")],
            2: [lambda: proj_qk1(3, 0, "q"), lambda: proj_qk1(3, 0, "k"),
                lambda: proj_v(4)],
            3: [lambda: proj_ml1(0, "q"), lambda: proj_ml1(0, "k"),
                lambda: proj_v(5)],
            4: [lambda: proj_qk1(0, 1, "q"), lambda: proj_qk1(0, 1, "k"),
                lambda: proj_v(6)],
            5: [lambda: proj_qk1(1, 1, "q"), lambda: proj_qk1(1, 1, "k"),
                lambda: proj_v(7)],
            6: [lambda: proj_qk1(2, 1, "q"), lambda: proj_qk1(2, 1, "k"),
                lambda: proj_v(8)],
            7: [lambda: proj_qk1(3, 1, "q"), lambda: proj_qk1(3, 1, "k")],
            8: [lambda: proj_ml1(1, "q"), lambda: proj_ml1(1, "k")],
            9: [lambda: proj_qk1(0, 2, "q"), lambda: proj_qk1(0, 2, "k")],
        }
        fillers1 = {
            0: [lambda: proj_qk1(1, 2, "q"), lambda: proj_qk1(1, 2, "k")],
            1: [lambda: proj_qk1(2, 2, "q"), lambda: proj_qk1(2, 2, "k")],
            2: [lambda: proj_qk1(3, 2, "q"), lambda: proj_qk1(3, 2, "k")],
            3: [lambda: proj_ml1(2, "q"), lambda: proj_ml1(2, "k")],
        }
        proj_qk1(0, 0, "q")
        proj_qk1(0, 0, "k")
        d0 = process_chunk(0, fillers0)
        for si, d in zip((2, 3, 4), d0):
            fillers1.setdefault(si, []).append(d)
        d1 = process_chunk(1, fillers1)
        fillers2 = {si: [d] for si, d in zip((2, 3, 4), d1)}
        d2 = process_chunk(2, fillers2, last=True)
        for d in d2:
            d()

        if DEBUG_TAPS:
            taps = dict(qT0=qT[0], kT0=kT[0], qml=qml, vx0=vext[0],
                        yTn4=yTn4, xt8=xt8_sb)
            for nm in DEBUG_TAPS:
                t = taps[nm]
                shp = [t.shape[0], t.shape[1]]
                dt_ = t[:].dtype
                dst = nc.dram_tensor(f"dbg_{nm}", shp, dt_,
                                     kind="ExternalOutput").ap()
                nc.sync.dma_start(dst[:, :], t[:])


# ----------------------------------------------------------------------------
# module build + run
# ----------------------------------------------------------------------------
_CACHE = {}


def _get_module():
    if "nc" in _CACHE:
        return _CACHE["nc"], _CACHE["plans"], _CACHE["mask_offs"], _CACHE["maskcat"]
    import concourse.tile as tile
    from concourse import bacc, mybir

    plans, maskcat, mask_offs = build_block_plan()
    eye = np.zeros((128, 2), np.float32)
    eye[0, 0] = eye[1, 1] = 1.0
    maskcat = (np.concatenate([maskcat, eye], axis=1)
               if maskcat.shape[1] else eye)
    mask_w = maskcat.shape[1]

    nc = bacc.Bacc("TRN2", target_bir_lowering=False, debug=False,
                   enable_asserts=False, num_devices=NCORES)
    f32 = mybir.dt.float32
    bf16 = mybir.dt.bfloat16
    f8 = mybir.dt.float8e4

    def din(name, shape, dt=f32):
        return nc.dram_tensor(name, list(shape), dt, kind="ExternalInput").ap()

    ins = dict(
        xt=din("xt", (EMBED, T), bf16),
        xt8=din("xt8", (128, 4 * T), f8),
        wqk=din("wqk", (128, 4096), f8),
        wv=din("wv", (128, 2048), bf16),
        wml=din("wml", (128, 1024), f8),
        wp=din("wp", (128, 2048), f32),
        consts=din("consts", (128, CONST_W), f32),
        masks=din("masks", (128, mask_w), bf16),
    )
    out_ap = nc.dram_tensor("out_p", [T, EMBED], f32, kind="ExternalOutput").ap()

    with tile.TileContext(nc) as tc:
        emit_kernel(tc, ins, out_ap, plans, mask_offs, mask_w)
    nc.compile()

    _CACHE.update(nc=nc, plans=plans, mask_offs=mask_offs, maskcat=maskcat)
    return nc, plans, mask_offs, maskcat


def build_in_maps(inputs):
    nc, plans, mask_offs, maskcat = _get_module()
    x = inputs["x"].astype(np.float32)
    cond = np.asarray(inputs["cond_mask"]).astype(np.int32)
    B = x.shape[0]
    assert B == NCORES, f"expected B={NCORES}, got {B}"

    ws = prep_weights(inputs)  # weights may differ between calls
    if "masks_bf" not in _CACHE:  # masks are static problem constants
        mc = maskcat if maskcat.shape[1] else np.zeros((128, 2), np.float32)
        _CACHE["masks_bf"] = mc.astype(BF16)
    perm, _ = build_perm()

    in_maps = []
    bias_cache = {}
    for b in range(B):
        cb = int(cond[b])
        if cb not in bias_cache:
            consts = ws["consts"].copy()
            consts[:, CONST_BIAS:CONST_BIAS + NSM] = core_biascols(inputs, cb)
            bias_cache[cb] = consts
        xtb = np.ascontiguousarray(x[b][perm].T)  # [512, T]
        xt8 = np.ascontiguousarray(
            xtb.reshape(4, 128, T).transpose(1, 0, 2).reshape(128, 4 * T)
        ).astype(F8)
        in_maps.append(dict(
            xt=xtb.astype(BF16), xt8=xt8,
            wqk=ws["wqk"], wv=ws["wv"], wml=ws["wml"], wp=ws["wp"],
            consts=bias_cache[cb], masks=_CACHE["masks_bf"],
        ))
    return nc, in_maps


def kernel(**inputs):
    from concourse import bass_utils

    inputs = {k: np.asarray(v) for k, v in inputs.items()}
    nc, in_maps = build_in_maps(inputs)
    res = bass_utils.run_bass_kernel_spmd(nc, in_maps, core_ids=list(range(NCORES)))
    _CACHE["last_results"] = res

    _, inv = build_perm()
    shift = host_const_shift(inputs)
    B = inputs["x"].shape[0]
    out = np.empty((B, T, EMBED), dtype=np.float32)
    for b in range(B):
        out[b] = res.results[b]["out_p"][inv] + shift
    return out



# revision 62
# speedup vs baseline: 1.0429x; 1.0244x over previous
"""Trainium2 Bass kernel for nn_CausalCrossConditionalSelfAttention.

Data-parallel over batch B=8, one element per core. Design:
  - fp8-e4m3 DoubleRow matmuls for the q/k/ml projections (weights scaled
    by SW into fp8's normal range, rescaled in the evacuation op); v and
    out-projection stay bf16 (fp8 v fails the error budget).
  - Scores per (softmax, 128-aligned query chunk): exact [zlo,zhi) block
    ranges, bin-packed into [128,1024] psum tiles, exp on ACT with the
    conditional CLIP-token bias as a per-partition operand, masks on
    DVE/Pool in bf16.
  - Transposed AV: out[queries<=128, 65] accumulates per (softmax, query
    group) in one-bank Y^T psum tiles (5 softmaxes x 65 cols per gen);
    the softmax denominator rides as a ones column of vext. Normalization
    is per-partition: [128,10] reciprocal + stride-0-broadcast muls in
    bf16 (f_mix folded into rz), so no Z broadcast matmuls or wide
    normalization ops exist.
  - y_norm -> yTn layout change via ONE xbar transpose DMA per query
    group into a compact [128, 4x128] tile (out[p, cc, t] = yn[t,
    cc*128+p]); out-projection reads it directly. 2-token tail goes
    through a PE transpose.
  - PSUM: 2x[128,1024] score tiles + a 4-buffer 1-bank pool shared by
    projections / Y^T gen tiles / out-proj (single-psum projections keep
    at most one transient alongside the 3 gen tiles).
  - Chunk-major schedule: per softmax, score waves -> fillers
    (projections for the next chunk) -> previous softmax's AV^T; gen
    evacuations on DVE (ACT for the last chunk); each chunk's
    normalization + out-projection defers into the next chunk's slots;
    merged per-trio output DMAs (immediate in the tail).
"""

import sys

if "/opt/trn_rl_repo" not in sys.path:
    sys.path.insert(0, "/opt/trn_rl_repo")

import numpy as np

try:
    import ml_dtypes
    BF16 = np.dtype(ml_dtypes.bfloat16)
    F8 = np.dtype(ml_dtypes.float8_e4m3)
except ImportError:  # pragma: no cover
    BF16 = None
    F8 = None

# fp8 weight scale: w*scale values (~0.0025) sit in e4m3's subnormal range,
# so store w*SW and multiply psum by 1/SW in the evacuation op.
SW = 256.0
DEBUG_TAPS = ()
DEBUG_QG = 1

# ----------------------------------------------------------------------------
# problem constants
# ----------------------------------------------------------------------------
BLOCK = 512
RECEP = 4
N_HEAD = 8
EMBED = 512
HS = 64
T = 2 * BLOCK + 2          # 1026
NSM = 10
NCORES = 8

# query chunks (offset, width): 128-aligned starts so the causal staircase's
# block zlo values never land inside a 128-query group (AV out base always 0)
ICS = [(0, 384), (384, 384), (768, 258)]
# key blocks (offset, height)
JBS = [(j * 128, 128) for j in range(8)] + [(1024, 2)]

# softmax id -> (mask kind, q/k source, v head)
SM_INFO = [
    (0, "loc", "main", 0), (1, "loc", "main", 1),
    (2, "seq", "main", 2), (3, "seq", "main", 3),
    (4, "seq", "main", 4), (5, "seq", "main", 5),
    (6, "seq", "main", 6), (7, "seq", "main", 7),
    (8, "loc", "ml", 2), (9, "loc", "ml", 3),
]
# softmax emission generations per chunk: Y^T psum gen tiles hold 5 softmaxes
# (5*65=325 cols, one bank); ytg staging column position of softmax s
GEN_SMS = [[0, 1, 2, 3, 4], [5, 6, 7, 8, 9]]
POS = {s: gi * 5 + i for gi, g in enumerate(GEN_SMS) for i, s in enumerate(g)}


def chunk_qgroups(ici):
    i0, W = ICS[ici]
    return [(g * 128, min(128, W - g * 128)) for g in range((W + 127) // 128)]


# ----------------------------------------------------------------------------
# host-side plan construction
# ----------------------------------------------------------------------------
def build_perm():
    perm = np.zeros(T, dtype=np.int64)
    perm[0], perm[1] = 0, 1
    b = np.arange(BLOCK)
    perm[2 + 2 * b] = 2 + b
    perm[3 + 2 * b] = 2 + BLOCK + b
    inv = np.argsort(perm)
    return perm, inv


def build_masks_orig():
    to = np.concatenate([np.zeros(2), np.arange(BLOCK) * 2 + 1, np.arange(BLOCK) * 2 + 2])
    seq = to[None, :] <= to[:, None]
    qo = np.concatenate([np.arange(BLOCK) * 2 + 1 - 2 * RECEP + 1] * 2)
    ko = np.concatenate([np.arange(BLOCK) * 2 + 1] * 2)
    de = ko[None, :] < qo[:, None]
    loc = seq.copy()
    loc[2:, 2:] = loc[2:, 2:] & (~de)
    return seq, loc


def build_block_plan():
    """Per (kind, ic): list of block dicts with exact column ranges.

    block = dict(jb, j0, rows, zlo, zhi, bias, mask=(mid,c0,c1) or None)
    Ordered so the first block covers [0, W) (widest) for PSUM start=True.
    """
    perm, _ = build_perm()
    seq, loc = build_masks_orig()
    Ms = seq[perm][:, perm]
    Ml = loc[perm][:, perm]

    mask_tiles = []
    tile_index = {}

    def tile_id(tile):
        key = tile.tobytes() + bytes(str(tile.shape), "ascii")
        if key not in tile_index:
            tile_index[key] = len(mask_tiles)
            mask_tiles.append(tile)
        return tile_index[key]

    plans = {}
    for kind, M in (("seq", Ms), ("loc", Ml)):
        plan = []
        for i0, W in ICS:
            blocks = []
            for jb, (j0, JH) in enumerate(JBS):
                sub = M[i0:i0 + W, j0:j0 + JH].T  # [JH, W] keys x queries
                if not sub.any():
                    continue
                nz_rows = np.flatnonzero(sub.any(axis=1))
                rows = int(nz_rows.max()) + 1
                colmask = sub[:rows].any(axis=0)
                nz_cols = np.flatnonzero(colmask)
                zlo, zhi = int(nz_cols.min()), int(nz_cols.max()) + 1
                core = sub[:rows, zlo:zhi]
                if core.all():
                    mask = None
                else:
                    pc = np.flatnonzero(~core.all(axis=0))
                    c0, c1 = zlo + int(pc.min()), zlo + int(pc.max()) + 1
                    mid = tile_id(
                        sub[:rows, c0:c1].astype(np.float32).copy())
                    mask = (mid, c0, c1)
                blocks.append(dict(jb=jb, j0=j0, rows=rows, zlo=zlo, zhi=zhi,
                                   bias=(j0 == 0), mask=mask))
            # widest-coverage block first (needed for PSUM start=True)
            blocks.sort(key=lambda b: (b["zlo"], -b["zhi"]))
            assert blocks[0]["zlo"] == 0 and blocks[0]["zhi"] == W, (kind, i0)
            plan.append(blocks)
        plans[kind] = plan

    offs, cat = [], []
    o = 0
    for t in mask_tiles:
        offs.append((o, t.shape[1]))
        cat.append(np.pad(t, ((0, 128 - t.shape[0]), (0, 0))))
        o += t.shape[1]
    maskcat = (np.concatenate(cat, axis=1) if cat
               else np.zeros((128, 0), np.float32))
    return plans, maskcat, offs


def build_exp_tiles(blocks, W):
    """Pack a chunk's blocks into [128,1024] score-psum tiles.

    Returns a list of tiles; each tile is a dict:
      placements: [(block, off)]          off in [0,1024), bank-contained
      exps: [("single", block, off)]      bias / tiny blocks
            [("run", [blocks], off, w)]   contiguous narrow blocks, one bank
            [("strided", [blocks], off0, stride, wmax)]
    """
    def bw(b):
        return b["zhi"] - b["zlo"]

    # tiny (rows<128) blocks pack as plain: their run-exps cover stale psum
    # rows, which downstream AVs never read (rows>=b["rows"] unused)
    specials = [b for b in blocks if b["bias"]]
    plain = sorted((b for b in blocks if not b["bias"]),
                   key=lambda b: b["jb"])
    tiles = []

    def new_tile():
        tiles.append(dict(placements=[], exps=[], used=0))
        return tiles[-1]

    if W <= 256:
        # uniform 256-wide slots, 4 per tile; strided exps over plain runs
        slots = specials + plain  # bias first, then jb order
        t = None
        for i, b in enumerate(slots):
            si = i % 4
            if si == 0:
                t = new_tile()
            t["placements"].append((b, si * 256))
        # exps: walk slots; specials single, plain grouped per tile
        for ti, t in enumerate(tiles):
            runb, ro, wmax = [], 0, 0
            for b, off in t["placements"]:
                if b["bias"] or b["rows"] < 128:
                    t["exps"].append(("single", b, off))
                else:
                    if not runb:
                        ro = off
                    runb.append(b)
                    wmax = max(wmax, bw(b))
            if runb:
                t["exps"].append(("strided", runb, ro, 256, wmax))
        return tiles

    wide = [b for b in plain if bw(b) > 256]
    narrow = [b for b in plain if bw(b) <= 256]
    # wide: stride-512 pairs occupying a full tile; narrow leftovers are
    # appended into pair spare bank space, extending the exp width (the
    # shorter bank's tail exps stale psum, which is never read downstream)
    pairs = []
    i = 0
    while i < len(wide):
        t = new_tile()
        pair = wide[i:i + 2]
        ext = []
        for g, b in enumerate(pair):
            t["placements"].append((b, g * 512))
            ext.append(bw(b))
        t["used"] = 2
        pairs.append((t, pair, ext))
        i += 2
    rem = []
    for b in narrow:
        placed = False
        for t, pair, ext in pairs:
            for k in sorted(range(len(pair)), key=lambda k: ext[k]):
                if ext[k] + bw(b) <= 512:
                    t["placements"].append((b, k * 512 + ext[k]))
                    ext[k] += bw(b)
                    placed = True
                    break
            if placed:
                break
        if not placed:
            rem.append(b)
    narrow = rem
    for t, pair, ext in pairs:
        if len(pair) == 2:
            t["exps"].append(("strided", pair, 0, 512, max(ext)))
        else:
            t["exps"].append(("run", pair, 0, ext[0]))

    free_banks = []
    def alloc_bank():
        if not free_banks:
            t = new_tile()
            t["used"] = 2
            free_banks.extend([(t, 0), (t, 512)])
        return free_banks.pop(0)

    if narrow:
        run, runw = [], 0
        bank = alloc_bank()
        for b in narrow:
            if runw + bw(b) > 512:
                t, boff = bank
                t["exps"].append(("run", run, boff, runw))
                bank = alloc_bank()
                run, runw = [], 0
            t, boff = bank
            t["placements"].append((b, boff + runw))
            run.append(b)
            runw += bw(b)
        t, boff = bank
        t["exps"].append(("run", run, boff, runw))
    for b in specials:
        bank = alloc_bank()
        t, boff = bank
        t["placements"].append((b, boff))
        t["exps"].append(("single", b, boff))
    return tiles


# ----------------------------------------------------------------------------
# host-side input prep
# ----------------------------------------------------------------------------
# consts tile layout (fp32, [128, CW]):
#   [0:4)   bq per m-chunk      [4:8) bk
#   [8]     bqml                [9]   bkml
#   [10:20) biascols (exp bias per softmax)
#   [20:30) f_s mix factor per softmax (all partitions; 1 except sm 2,3,8,9)
CONST_BQ, CONST_BK, CONST_BQML, CONST_BKML = 0, 4, 8, 9
CONST_BIAS = 10
CONST_F = 20
CONST_W = 30


def prep_weights(w):
    """Shared (per-batch-invariant) device buffers."""
    f = np.float32
    scale = f(1.0 / np.sqrt(HS))

    wqT = w["w_query"].astype(f).T * scale     # [cin, cout]
    wkT = w["w_key"].astype(f).T
    wvT = w["w_value"].astype(f).T
    wpT = w["w_proj"].astype(f).T
    wqmlT = w["w_query_ml"].astype(f).T * scale  # [512, 128]
    wkmlT = w["w_key_ml"].astype(f).T

    # wqk8: fp8 DoubleRow layout [128, kc(4), 1024] -> [128, 4096]
    # [p, kc, c] = (wq|wk).T[kc*128+p, c] * SW
    wqk = np.ascontiguousarray(
        (np.concatenate([wqT, wkT], axis=1) * SW)
        .reshape(4, 128, 1024).transpose(1, 0, 2).reshape(128, 4096)
    ).astype(F8)
    # wv single tile [128, 4*512]: [p, kc*512+c] = wvT[kc*128+p, c]
    wv = np.ascontiguousarray(
        wvT.reshape(4, 128, 512).transpose(1, 0, 2).reshape(128, 2048)
    ).astype(BF16)
    # wml8 fp8 DR tile [128, 4*256]: per kc [qml 128 | kml 128]
    wml = np.ascontiguousarray(
        (np.concatenate([wqmlT.reshape(4, 128, 128),
                         wkmlT.reshape(4, 128, 128)], axis=2) * SW)
        .transpose(1, 0, 2).reshape(128, 1024)
    ).astype(F8)
    # wp bf16 single tile [128, 4*512] (pairs with bf16 yTn in out-proj)
    wp = np.ascontiguousarray(
        wpT.reshape(4, 128, 512).transpose(1, 0, 2).reshape(128, 2048)
    ).astype(BF16)

    # consts (biascols filled per core)
    consts = np.zeros((128, CONST_W), dtype=f)
    consts[:, CONST_BQ:CONST_BQ + 4] = (w["b_query"].astype(f) * scale
                                        ).reshape(4, 128).T
    consts[:, CONST_BK:CONST_BK + 4] = w["b_key"].astype(f).reshape(4, 128).T
    consts[:, CONST_BQML] = (w["b_query_ml"].astype(f) * scale)
    consts[:, CONST_BKML] = w["b_key_ml"].astype(f)

    wg = w["w_mix"].astype(f)[:, 0, 0, 0]
    wl = w["w_mix"].astype(f)[:, 1, 0, 0]
    fs = np.ones(NSM, dtype=f)
    fs[2], fs[3] = wg[0], wg[1]
    fs[8], fs[9] = wl[0], wl[1]
    consts[:, CONST_F:CONST_F + NSM] = fs[None, :]
    return dict(wqk=wqk, wv=wv, wml=wml, wp=wp, consts=consts)


def core_biascols(w, cond_b):
    f = np.float32
    bias = np.zeros((128, NSM), dtype=f)
    if cond_b > 0:
        clip8 = np.maximum(w["att_bias_clip"].astype(f)[0, :, 0], 0.0) * 10.0
        clip2 = np.maximum(w["att_bias_clip_ml"].astype(f)[0, :, 0], 0.0) * 10.0
        bias[1, :N_HEAD] = clip8
        bias[1, N_HEAD:] = clip2
    return bias


def host_const_shift(w):
    bv = w["b_value"].astype(np.float64)
    wg = w["w_mix"].astype(np.float64)[:, 0, 0, 0]
    wl = w["w_mix"].astype(np.float64)[:, 1, 0, 0]
    scale_h = np.ones(N_HEAD)
    scale_h[2] = wg[0] + wl[0]
    scale_h[3] = wg[1] + wl[1]
    yshift = (bv.reshape(N_HEAD, HS) * scale_h[:, None]).reshape(-1)
    return (yshift @ w["w_proj"].astype(np.float64).T
            + w["b_proj"].astype(np.float64)).astype(np.float32)


# ----------------------------------------------------------------------------
# bass kernel emission
# ----------------------------------------------------------------------------
def emit_kernel(tc, ins, out_ap, plans, mask_offs, mask_w):
    from contextlib import ExitStack
    from concourse import mybir

    nc = tc.nc
    f32 = mybir.dt.float32
    f32r = mybir.dt.float32r
    bf16 = mybir.dt.bfloat16
    AF = mybir.ActivationFunctionType

    def r(ap):
        return ap.bitcast(f32r)

    with ExitStack() as ctx:
        P = ctx.enter_context(tc.tile_pool(name="persist", bufs=1))

        # ---------------- persistent SBUF tiles ----------------
        f8 = mybir.dt.float8e4
        xT = [P.tile([128, T], bf16, name=f"x{k}", tag=f"x{k}") for k in range(4)]
        xt8_sb = P.tile([128, 4 * T], f8, name="xt8", tag="xt8")
        wqk8_sb = P.tile([128, 4096], f8, name="wqk8", tag="wqk8")
        wv_sb = P.tile([128, 2048], bf16, name="wv", tag="wv")
        wml_sb = P.tile([128, 1024], f8, name="wml", tag="wml")
        wp_sb = P.tile([128, 2048], bf16, name="wp", tag="wp")
        consts = P.tile([128, CONST_W], f32, name="consts", tag="consts")
        maskcat = P.tile([128, mask_w], bf16, name="maskcat", tag="maskcat")
        # DoubleRow-ready views [p, kc, cols]
        x8v = xt8_sb[:].rearrange("p (k c) -> p k c", c=T)
        w8v = wqk8_sb[:].rearrange("p (k c) -> p k c", c=1024)
        wml8v = wml_sb[:].rearrange("p (k c) -> p k c", c=256)
        DR = mybir.MatmulPerfMode.DoubleRow

        qT = [P.tile([128, T], bf16, name=f"qT{m}", tag=f"qT{m}") for m in range(4)]
        kT = [P.tile([128, T], bf16, name=f"kT{m}", tag=f"kT{m}") for m in range(4)]
        qml = P.tile([128, T], bf16, name="qml", tag="qml")
        kml = P.tile([128, T], bf16, name="kml", tag="kml")
        vext = [P.tile([128, N_HEAD * 65], bf16, name=f"vx{t}", tag=f"vx{t}")
                for t in range(9)]


        # ---------------- DMA loads ----------------
        # All on the SP queue (HWDGE/DMA-device serialize transfers anyway;
        # keeping ACT's sequencer free for exps). Order = need order.
        nc.sync.dma_start(wqk8_sb[:], ins["wqk"][:, :])
        nc.sync.dma_start(xt8_sb[:, 0:2 * T], ins["xt8"][:, 0:2 * T])
        nc.sync.dma_start(r(consts[:]), r(ins["consts"][:, :]))
        nc.sync.dma_start(xt8_sb[:, 2 * T:4 * T], ins["xt8"][:, 2 * T:4 * T])
        nc.sync.dma_start(maskcat[:], ins["masks"][:, :])
        nc.sync.dma_start(xT[0][:], ins["xt"][0:128, :])
        nc.sync.dma_start(xT[1][:], ins["xt"][128:256, :])
        nc.sync.dma_start(wv_sb[:], ins["wv"][:, :])
        nc.sync.dma_start(xT[2][:], ins["xt"][256:384, :])
        nc.sync.dma_start(xT[3][:], ins["xt"][384:512, :])
        nc.sync.dma_start(wml_sb[:], ins["wml"][:, :])
        nc.sync.dma_start(wp_sb[:], ins["wp"][:, :])

        # ones columns for the Z row of every AV matmul
        for tt in range(9):
            vx = vext[tt][:].rearrange("p (h e) -> p h e", e=65)
            nc.gpsimd.memset(vx[:, :, 64:65], 1.0)
        # 2x2 identity (tail-transpose operand) rides in the masks buffer
        eye2 = maskcat[0:2, mask_w - 2:mask_w]
        # bf16 copy of the mix factors (pairs with bf16 rz in the norm)
        fcol_bf = P.tile([128, 16], bf16, name="fcol", tag="fcol")
        nc.vector.tensor_copy(fcol_bf[:, 0:NSM],
                              consts[:, CONST_F:CONST_F + NSM])

        # tile pools (SBUF work tiles)
        ptp = ctx.enter_context(tc.tile_pool(name="ptp", bufs=8))
        ytgp = ctx.enter_context(tc.tile_pool(name="ytgp", bufs=4))   # [128,650] f32
        ynp = ctx.enter_context(tc.tile_pool(name="ynp", bufs=4))     # [128,512] bf16
        mltp = ctx.enter_context(tc.tile_pool(name="mltp", bufs=3))
        rzp = ctx.enter_context(tc.tile_pool(name="rzp", bufs=3))
        ytqp = ctx.enter_context(tc.tile_pool(name="ytqp", bufs=3))

        # psum pools: sp (2 x [128,1024] score tiles = 4 banks) + genp
        # (3 x 1-bank long-lived Y^T gen tiles) + smallp (1 bank rotating
        # through transient projection/out-proj/tail tiles) = 8 banks.
        # Long-lived and transient tiles MUST NOT share a pool: rotation
        # could hand a transient a buffer owned by a live gen tile, putting
        # a PE instruction ahead of the AVs that free it (deadlock).
        sp = ctx.enter_context(tc.tile_pool(name="sp", bufs=2, space="PSUM"))
        wp4 = ctx.enter_context(tc.tile_pool(name="wp4", bufs=4, space="PSUM"))
        genp = wp4
        smallp = wp4

        def alloc_score():
            return sp.tile([128, 1024], f32, name="sp", tag="sp")

        def alloc_small():
            return smallp.tile([128, 512], f32, name="smallp", tag="wp4")

        # ---------------- emission helpers ----------------
        MUL, ADD = mybir.AluOpType.mult, mybir.AluOpType.add

        def evac(dst, ps_ap, bcol):
            """psum -> sbuf bf16 with 1/SW rescale + bias add."""
            nc.vector.tensor_scalar(dst, ps_ap, 1.0 / SW,
                                    consts[:, bcol:bcol + 1],
                                    op0=MUL, op1=ADD)

        def proj_qk1(m, ici, which):
            """q or k projection for head-pair m, query chunk ici (fp8 DR).

            Single-psum so the shared wp4 pool holds at most one projection
            tile at a time alongside the three Y^T gen tiles."""
            i0, W = ICS[ici]
            coff = 0 if which == "q" else 512
            ps = alloc_small()
            for j in range(2):
                nc.tensor.matmul(
                    ps[:, 0:W],
                    lhsT=w8v[:, 2 * j:2 * j + 2,
                             coff + m * 128:coff + (m + 1) * 128],
                    rhs=x8v[:, 2 * j:2 * j + 2, i0:i0 + W],
                    start=(j == 0), stop=(j == 1), perf_mode=DR)
            dst_t = qT if which == "q" else kT
            bcol = (CONST_BQ if which == "q" else CONST_BK) + m
            evac(dst_t[m][:, i0:i0 + W], ps[:, 0:W], bcol)

        def proj_ml1(ici, which):
            i0, W = ICS[ici]
            coff, bcol = ((0, CONST_BQML) if which == "q"
                          else (128, CONST_BKML))
            ps = alloc_small()
            for j in range(2):
                nc.tensor.matmul(
                    ps[:, 0:W],
                    lhsT=wml8v[:, 2 * j:2 * j + 2, coff:coff + 128],
                    rhs=x8v[:, 2 * j:2 * j + 2, i0:i0 + W],
                    start=(j == 0), stop=(j == 1), perf_mode=DR)
            dst = (qml if which == "q" else kml)[:, i0:i0 + W]
            evac(dst, ps[:, 0:W], bcol)

        def proj_v(tt):
            j0, JH = JBS[tt]
            ps = alloc_small()
            for kc in range(4):
                nc.tensor.matmul(
                    ps[0:JH, :],
                    lhsT=xT[kc][:, j0:j0 + JH],
                    rhs=wv_sb[:, kc * 512:(kc + 1) * 512],
                    start=(kc == 0), stop=(kc == 3))
            vx = vext[tt][0:JH].rearrange("p (h e) -> p h e", e=65)
            nc.scalar.activation(
                vx[:, :, 0:64], ps[0:JH, :].rearrange("p (h d) -> p h d", d=64),
                AF.Copy)

        class Chunk:
            """One (softmax, query-chunk): score waves -> per-qgroup AV^T."""

            def __init__(self, s, ici):
                self.s, self.ici = s, ici
                _, self.kind, src_, self.hv = SM_INFO[s]
                self.i0, self.W = ICS[ici]
                if src_ == "main":
                    self.qt, self.kt = qT[s // 2], kT[s // 2]
                    self.off = (s % 2) * 64
                else:
                    self.qt, self.kt, self.off = qml, kml, (s - N_HEAD) * 64
                self.blocks = plans[self.kind][ici]
                self.tiles = build_exp_tiles(self.blocks, self.W)
                self.n_waves = len(self.tiles)
                self.pts = {}

            def score_wave(self, w):
                """One psum tile: its score matmuls, exps, and masks."""
                i0, s = self.i0, self.s
                tile = self.tiles[w]
                st = alloc_score()
                pt = ptp.tile([128, 1024], bf16, name="pt", tag="pt")
                for b, off in tile["placements"]:
                    bwid = b["zhi"] - b["zlo"]
                    nc.tensor.matmul(
                        st[0:b["rows"], off:off + bwid],
                        lhsT=self.kt[self.off:self.off + 64,
                                     b["j0"]:b["j0"] + b["rows"]],
                        rhs=self.qt[self.off:self.off + 64,
                                    i0 + b["zlo"]:i0 + b["zhi"]],
                        start=True, stop=True)
                    self.pts[b["jb"]] = (pt, off, b)
                for exp in tile["exps"]:
                    if exp[0] == "single":
                        _, b, off = exp
                        rows, bwid = b["rows"], b["zhi"] - b["zlo"]
                        if b["bias"]:
                            nc.scalar.activation(
                                pt[0:rows, off:off + bwid],
                                st[0:rows, off:off + bwid], AF.Exp,
                                bias=consts[0:rows,
                                            CONST_BIAS + s:CONST_BIAS + s + 1],
                                scale=1.0)
                        else:
                            nc.scalar.activation(
                                pt[0:rows, off:off + bwid],
                                st[0:rows, off:off + bwid], AF.Exp)
                    elif exp[0] == "run":
                        _, blks, off, wtot = exp
                        nc.scalar.activation(
                            pt[:, off:off + wtot], st[:, off:off + wtot],
                            AF.Exp)
                    else:  # strided
                        _, blks, off0, stride, wmax = exp
                        s0, ng = off0 // stride, len(blks)
                        nc.scalar.activation(
                            pt[:].rearrange("p (g c) -> p g c", c=stride)
                            [:, s0:s0 + ng, 0:wmax],
                            st[:].rearrange("p (g c) -> p g c", c=stride)
                            [:, s0:s0 + ng, 0:wmax],
                            AF.Exp)
                for b, off in tile["placements"]:
                    if b["mask"] is not None:
                        mid, c0, c1 = b["mask"]
                        mo, mw = mask_offs[mid]
                        mask_rr[0] += 1
                        if self.kind == "seq":
                            eng = (nc.gpsimd if mask_rr[0] % 4 == 0
                                   else nc.vector)
                        else:  # alternate loc masks DVE/Pool
                            eng = (nc.gpsimd if mask_rr[0] % 2
                                   else nc.vector)
                        o0 = off + c0 - b["zlo"]
                        eng.tensor_mul(
                            pt[0:b["rows"], o0:o0 + mw],
                            pt[0:b["rows"], o0:o0 + mw],
                            maskcat[0:b["rows"], mo:mo + mw])

            def av_qgroup(self, glo, rows_qg, yt, pos):
                """Accumulate this softmax's AV^T for chunk-relative queries
                [glo, glo+rows_qg) into yt psum cols [pos*65, pos*65+65).

                Output partitions are queries; column 64-of-65 collects the
                softmax denominator via the ones column in vext. blocks[0]
                covers [0, W) so the start=True matmul spans all rows; later
                (partial) blocks always satisfy zlo <= glo (staircase aligns
                with the 128 query grid) and accumulate row subranges."""
                ghi = glo + rows_qg
                blks = [b for b in self.blocks
                        if max(b["zlo"], glo) < min(b["zhi"], ghi)]
                for bi, b in enumerate(blks):
                    assert b["zlo"] <= glo, (self.s, self.ici, glo, b["zlo"])
                    hi = min(b["zhi"], ghi)
                    pt, off, _ = self.pts[b["jb"]]
                    nc.tensor.matmul(
                        yt[0:hi - glo, pos * 65:pos * 65 + 65],
                        lhsT=pt[0:b["rows"],
                                off + glo - b["zlo"]:off + hi - b["zlo"]],
                        rhs=vext[b["jb"]][0:b["rows"],
                                          self.hv * 65:self.hv * 65 + 65],
                        start=(bi == 0), stop=(bi == len(blks) - 1))

        mask_rr = [0]

        # merged output staging: one tile per trio of token chunks
        ost3 = [P.tile([128, 1536], f32, name=f"ost{i}", tag=f"ost{i}")
                for i in range(3)]

        def out_proj(m, ytq, JHt):
            """Out-projection for token chunk m (= query group m).

            ytq: compact transposed tile [128, cc(4), JHt] (c = cc*128+p)."""
            j0, JH = JBS[m]
            yqv = ytq[:].rearrange("p (c t) -> p c t", t=JHt)
            trio, slot = divmod(m, 3)
            po = alloc_small()
            for p in range(4):
                nc.tensor.matmul(
                    po[0:JH, :],
                    lhsT=yqv[:, p, 0:JH],
                    rhs=wp_sb[:, p * 512:(p + 1) * 512],
                    start=(p == 0), stop=(p == 3))
            dst = ost3[trio][0:JH, slot * 512:slot * 512 + 512]
            if trio == 2:  # tail: ACT is drained, DVE is not
                nc.scalar.activation(dst, po[0:JH, :], AF.Copy)
            else:
                nc.vector.tensor_copy(dst, po[0:JH, :])
            if trio == 2:  # final trio: DMA each block immediately (tail)
                nc.sync.dma_start(
                    out_ap[j0:j0 + JH, :],
                    ost3[trio][0:JH, slot * 512:slot * 512 + 512])
            elif slot == 2:  # trio complete -> one merged DMA
                t0 = trio * 384
                ov = ost3[trio][:].rearrange("p (s c) -> p s c", c=512)
                nc.sync.dma_start(
                    out_ap[t0:t0 + 384, :].rearrange("(s p) c -> p s c", s=3),
                    ov[:, 0:3])

        def norm_qg(ici, glo, rows_qg, ytg, qg_global):
            """Normalize one query group from its ytg staging and fill yTn.

            rz[:, s] = f_s / Z_s per query partition; y_norm = ytg * rz
            broadcast; ml components scaled by w_l are added into mixed
            heads 2/3; yTn gets the [c, token] layout via DMA transpose
            (PE transpose for the 2-token tail)."""
            ytgv = ytg[0:rows_qg].rearrange("p (s e) -> p s e", e=65)
            rz = rzp.tile([128, 16], bf16, name="rz", tag="rz")
            ctx2 = nc.allow_low_precision(reason="bf16 softmax normalization")
            ctx2.__enter__()
            nc.vector.reciprocal(rz[0:rows_qg, 0:NSM], ytgv[:, :, 64])
            nc.vector.tensor_tensor(
                rz[0:rows_qg, 0:NSM], rz[0:rows_qg, 0:NSM],
                fcol_bf[0:rows_qg, 0:NSM], op=MUL)
            yn = ynp.tile([128, 512], bf16, name="yn", tag="yn")
            mlt = mltp.tile([128, 128], bf16, name="mlt", tag="mlt")
            ynv = yn[0:rows_qg].rearrange("p (s e) -> p s e", e=64)
            mlv = mlt[0:rows_qg].rearrange("p (s e) -> p s e", e=64)
            nc.vector.tensor_tensor(
                ynv[:, 0:8], ytgv[:, 0:8, 0:64],
                rz[0:rows_qg, 0:8, None].broadcast_to((rows_qg, 8, 64)),
                op=MUL)
            # final chunk (qg 6-8): Pool is idle in the drain tail; taking
            # the ml ops off DVE shortens the serial normalization chain
            veng = nc.gpsimd if qg_global >= 6 else nc.vector
            veng.tensor_tensor(
                mlv[:, 0:2], ytgv[:, 8:10, 0:64],
                rz[0:rows_qg, 8:10, None].broadcast_to((rows_qg, 2, 64)),
                op=MUL)
            veng.tensor_tensor(yn[0:rows_qg, 128:192],
                               yn[0:rows_qg, 128:192],
                               mlt[0:rows_qg, 0:64], op=ADD)
            veng.tensor_tensor(yn[0:rows_qg, 192:256],
                               yn[0:rows_qg, 192:256],
                               mlt[0:rows_qg, 64:128], op=ADD)
            if DEBUG_TAPS and qg_global == DEBUG_QG:
                dbg_ytg = P.tile([128, 650], bf16, name="dytg", tag="dytg")
                dbg_yn = P.tile([128, 512], bf16, name="dyn", tag="dyn")
                nc.vector.tensor_copy(dbg_ytg[0:rows_qg, :], ytg[0:rows_qg, :])
                nc.vector.tensor_copy(dbg_yn[0:rows_qg, :], yn[0:rows_qg, :])
                for nm, t in (("ytgq", dbg_ytg), ("ynq", dbg_yn)):
                    dst = nc.dram_tensor(f"dbg_{nm}", [128, t.shape[1]],
                                         t[:].dtype, kind="ExternalOutput").ap()
                    nc.sync.dma_start(dst[:, :], t[:])
            if rows_qg >= 16:
                # one transpose DMA -> compact [128, 4, rows] tile
                # (out[p, cc, t] = yn[t, cc*128+p]; out must be contiguous)
                ytq = ytqp.tile([128, 512], bf16, name="ytq", tag="ytq")
                nc.sync.dma_start_transpose(
                    ytq[:].rearrange("p (c t) -> p c t", t=rows_qg),
                    yn[0:rows_qg, :])
                out_proj(qg_global, ytq, rows_qg)
            else:  # 2-token tail: PE transpose through a bf16 psum tile
                tps = wp4.tile([128, 1024], bf16, name="tp", tag="wp4")
                for cc in range(4):
                    nc.tensor.transpose(
                        tps[:, cc * 2:cc * 2 + 2],
                        yn[0:rows_qg, cc * 128:(cc + 1) * 128], eye2)
                ytq = ytqp.tile([128, 512], bf16, name="ytq", tag="ytq")
                nc.vector.tensor_copy(ytq[:, 0:4 * rows_qg], tps[:, 0:8])
                out_proj(qg_global, ytq, rows_qg)
            ctx2.__exit__(None, None, None)

        def process_chunk(ici, fillers, last=False):
            """All 10 softmaxes of one query chunk, in two 5-softmax gens.

            Per softmax: score waves -> (fillers) -> next chunk's waves for
            the same softmax (prebuild: fills ACT gaps early so the final
            chunk is AV/norm-only) -> previous softmax's AV^T. Gen g's Y^T
            psum tiles (one bank per qgroup) evacuate into ytg when the
            gen's last softmax has AV'd (on ACT for the last chunk, where
            ACT is otherwise drained)."""
            qgs = chunk_qgroups(ici)
            base_qg = sum(len(chunk_qgroups(i)) for i in range(ici))
            ytg_t = [ytgp.tile([128, 650], bf16, name="ytg", tag="ytg")
                     for _ in qgs]
            gen_tiles = {}
            pend = None

            def flush(pend_ch):
                ch, gi = pend_ch
                if gi not in gen_tiles:
                    gen_tiles[gi] = [
                        wp4.tile([128, 512], f32, name="yt", tag="wp4")
                        for _ in qgs]
                for qi, (glo, rows_qg) in enumerate(qgs):
                    ch.av_qgroup(glo, rows_qg, gen_tiles[gi][qi],
                                 POS[ch.s] % 5)
                if ch.s == GEN_SMS[gi][-1]:  # gen complete -> evacuate
                    with nc.allow_low_precision(reason="bf16 ytg staging"):
                        for qi, (glo, rows_qg) in enumerate(qgs):
                            dst = ytg_t[qi][0:rows_qg,
                                            gi * 325:gi * 325 + 325]
                            src = gen_tiles[gi][qi][0:rows_qg, 0:325]
                            if last:
                                nc.scalar.activation(dst, src, AF.Copy)
                            else:
                                nc.vector.tensor_copy(dst, src)

            for gi, sms in enumerate(GEN_SMS):
                for s in sms:
                    si = POS[s]
                    ch = Chunk(s, ici)
                    for w in range(ch.n_waves):
                        ch.score_wave(w)
                    for f in fillers.get(si, []):
                        f()
                    if pend is not None:
                        flush(pend)
                    pend = (ch, gi)
            flush(pend)
            # defer norms/out-projs into the next chunk's slots so their
            # psum/pool allocations trail the next chunk's gen tiles
            return [(lambda glo=glo, rows_qg=rows_qg, t=t, q=q:
                     norm_qg(ici, glo, rows_qg, t, q))
                    for (glo, rows_qg), t, q in
                    zip(qgs, ytg_t, range(base_qg, base_qg + len(qgs)))]

        # ---------------- emission schedule ----------------
        # Chunk-major. Projections for chunk ici+1 ride as fillers inside
        # chunk ici; all of chunk 0's own projections are emitted up front /
        # in its first softmax slots (DMA-gated anyway).
        fillers0 = {
            0: [lambda: proj_v(0), lambda: proj_v(1),
                lambda: proj_qk1(1, 0, "q"), lambda: proj_qk1(1, 0, "k")],
            1: [lambda: proj_v(2), lambda: proj_v(3),
                lambda: proj_qk1(2, 0, "q"), lambda: proj_qk1(2, 0, "k")],
            2: [lambda: proj_qk1(3, 0, "q"), lambda: proj_qk1(3, 0, "k"),
                lambda: proj_v(4)],
            3: [lambda: proj_ml1(0, "q"), lambda: proj_ml1(0, "k"),
                lambda: proj_v(5)],
            4: [lambda: proj_qk1(0, 1, "q"), lambda: proj_qk1(0, 1, "k"),
                lambda: proj_v(6)],
            5: [lambda: proj_qk1(1, 1, "q"), lambda: proj_qk1(1, 1, "k"),
                lambda: proj_v(7)],
            6: [lambda: proj_qk1(2, 1, "q"), lambda: proj_qk1(2, 1, "k"),
                lambda: proj_v(8)],
            7: [lambda: proj_qk1(3, 1, "q"), lambda: proj_qk1(3, 1, "k")],
            8: [lambda: proj_ml1(1, "q"), lambda: proj_ml1(1, "k")],
            9: [lambda: proj_qk1(0, 2, "q"), lambda: proj_qk1(0, 2, "k")],
        }
        fillers1 = {
            0: [lambda: proj_qk1(1, 2, "q"), lambda: proj_qk1(1, 2, "k")],
            1: [lambda: proj_qk1(2, 2, "q"), lambda: proj_qk1(2, 2, "k")],
            2: [lambda: proj_qk1(3, 2, "q"), lambda: proj_qk1(3, 2, "k")],
            3: [lambda: proj_ml1(2, "q"), lambda: proj_ml1(2, "k")],
        }
        proj_qk1(0, 0, "q")
        proj_qk1(0, 0, "k")
        d0 = process_chunk(0, fillers0)
        for si, d in zip((2, 3, 4), d0):
            fillers1.setdefault(si, []).append(d)
        d1 = process_chunk(1, fillers1)
        fillers2 = {si: [d] for si, d in zip((2, 3, 4), d1)}
        d2 = process_chunk(2, fillers2, last=True)
        for d in d2:
            d()

        if DEBUG_TAPS:
            taps = dict(qT0=qT[0], kT0=kT[0], qml=qml, vx0=vext[0],
                        yTn4=yTn4, xt8=xt8_sb)
            for nm in DEBUG_TAPS:
                t = taps[nm]
                shp = [t.shape[0], t.shape[1]]
                dt_ = t[:].dtype
                dst = nc.dram_tensor(f"dbg_{nm}", shp, dt_,
                                     kind="ExternalOutput").ap()
                nc.sync.dma_start(dst[:, :], t[:])


# ----------------------------------------------------------------------------
# module build + run
# ----------------------------------------------------------------------------
_CACHE = {}


def _get_module():
    if "nc" in _CACHE:
        return _CACHE["nc"], _CACHE["plans"], _CACHE["mask_offs"], _CACHE["maskcat"]
    import concourse.tile as tile
    from concourse import bacc, mybir

    plans, maskcat, mask_offs = build_block_plan()
    eye = np.zeros((128, 2), np.float32)
    eye[0, 0] = eye[1, 1] = 1.0
    maskcat = (np.concatenate([maskcat, eye], axis=1)
               if maskcat.shape[1] else eye)
    mask_w = maskcat.shape[1]

    nc = bacc.Bacc("TRN2", target_bir_lowering=False, debug=False,
                   enable_asserts=False, num_devices=NCORES)
    f32 = mybir.dt.float32
    bf16 = mybir.dt.bfloat16
    f8 = mybir.dt.float8e4

    def din(name, shape, dt=f32):
        return nc.dram_tensor(name, list(shape), dt, kind="ExternalInput").ap()

    ins = dict(
        xt=din("xt", (EMBED, T), bf16),
        xt8=din("xt8", (128, 4 * T), f8),
        wqk=din("wqk", (128, 4096), f8),
        wv=din("wv", (128, 2048), bf16),
        wml=din("wml", (128, 1024), f8),
        wp=din("wp", (128, 2048), f32),
        consts=din("consts", (128, CONST_W), f32),
        masks=din("masks", (128, mask_w), bf16),
    )
    out_ap = nc.dram_tensor("out_p", [T, EMBED], f32, kind="ExternalOutput").ap()

    with tile.TileContext(nc) as tc:
        emit_kernel(tc, ins, out_ap, plans, mask_offs, mask_w)
    nc.compile()

    _CACHE.update(nc=nc, plans=plans, mask_offs=mask_offs, maskcat=maskcat)
    return nc, plans, mask_offs, maskcat


def build_in_maps(inputs):
    nc, plans, mask_offs, maskcat = _get_module()
    x = inputs["x"].astype(np.float32)
    cond = np.asarray(inputs["cond_mask"]).astype(np.int32)
    B = x.shape[0]
    assert B == NCORES, f"expected B={NCORES}, got {B}"

    ws = prep_weights(inputs)  # weights may differ between calls
    if "masks_bf" not in _CACHE:  # masks are static problem constants
        mc = maskcat if maskcat.shape[1] else np.zeros((128, 2), np.float32)
        _CACHE["masks_bf"] = mc.astype(BF16)
    perm, _ = build_perm()

    in_maps = []
    bias_cache = {}
    for b in range(B):
        cb = int(cond[b])
        if cb not in bias_cache:
            consts = ws["consts"].copy()
            consts[:, CONST_BIAS:CONST_BIAS + NSM] = core_biascols(inputs, cb)
            bias_cache[cb] = consts
        xtb = np.ascontiguousarray(x[b][perm].T)  # [512, T]
        xt8 = np.ascontiguousarray(
            xtb.reshape(4, 128, T).transpose(1, 0, 2).reshape(128, 4 * T)
        ).astype(F8)
        in_maps.append(dict(
            xt=xtb.astype(BF16), xt8=xt8,
            wqk=ws["wqk"], wv=ws["wv"], wml=ws["wml"], wp=ws["wp"],
            consts=bias_cache[cb], masks=_CACHE["masks_bf"],
        ))
    return nc, in_maps


def kernel(**inputs):
    from concourse import bass_utils

    inputs = {k: np.asarray(v) for k, v in inputs.items()}
    nc, in_maps = build_in_maps(inputs)
    res = bass_utils.run_bass_kernel_spmd(nc, in_maps, core_ids=list(range(NCORES)))
    _CACHE["last_results"] = res

    _, inv = build_perm()
    shift = host_const_shift(inputs)
    B = inputs["x"].shape[0]
    out = np.empty((B, T, EMBED), dtype=np.float32)
    for b in range(B):
        out[b] = res.results[b]["out_p"][inv] + shift
    return out

